# revision 8
# baseline (speedup 1.0000x reference)
"""Trainium2 Bass kernel for a dense transformer block (pre-LN attn + MLP).

B=4, T=2048, D=768, H=12 (DH=64), DFF=3072, fp32.

Sharding: 8 cores = 4 batches x 2 roles. Each core processes one batch and
owns 1024 query tokens (two 512-blocks, paired {0,3}/{1,2} for causal load
balance). K/V are computed for the full 2048 tokens on both cores of a batch
(cheap), so there are NO collectives.

SPMD uniformity: all 8 cores run ONE identical NEFF. Causal structure is
carried in DATA, not code:
  - host permutes each batch's token axis to [own0, own1, otherA, otherB]
  - q-slot0 attends s-chunks {0..3, 8..11}; q-slot1 attends s-chunks {0..15}
  - per-(slot,chunk) exp scale/bias inputs select live / dead (zero) chunks
  - 4 canonical triangular masks handle the self-diagonal 512-blocks

Everything on-chip runs in a transposed layout (features on partitions,
tokens on the free axis) so no on-chip transposes are needed; all weight /
input transposes happen on the host in numpy. Matmuls run as float32r
(full PE speed, ~bf16x2 precision). LayerNorm statistics are computed with
ones-column matmuls; per-token stats are broadcast across partitions with
K=1 outer-product matmuls. Softmax denominators come for free from a ones
column appended to V (65-row PV matmul); the divide is folded in after PV.
"""

import sys

sys.path.insert(0, "/opt/trn_rl_repo")

from contextlib import ExitStack

import numpy as np

import concourse.bass as bass
import concourse.mybir as mybir
import concourse.tile as tile
from concourse import bacc
from concourse.bass_utils import run_bass_kernel_spmd

F32 = mybir.dt.float32
F32R = mybir.dt.float32r
AF = mybir.ActivationFunctionType
ALU = mybir.AluOpType

H, D, DFF = 12, 768, 3072
DH = 64
B, T = 4, 2048
EPS = 1e-5
P = 128
NC = D // P          # 6 feature chunks
NF = DFF // P        # 24 ff tiles
TB = 512             # token block
NTB = T // TB        # 4 blocks
SLOT_CHUNKS = [[0, 1, 2, 3, 8, 9, 10, 11], list(range(16))]
# role -> permuted block order [own0, own1, restA, restB] (original block ids)
ROLE_ORDER = [[0, 3, 1, 2], [1, 2, 0, 3]]
DEAD = -30000.0      # exp(DEAD) == 0 in fp32

_cached = {}


def _build_nc():
    nc = bacc.Bacc("TRN2", target_bir_lowering=False, debug=False,
                   enable_asserts=False, num_devices=8)

    def din(name, shape, dt=F32R):
        return nc.dram_tensor(name, shape, dt, kind="ExternalInput").ap()

    xt_d = din("xt", [D, T])                 # X[b].T, token-permuted
    wqt_d = din("wqt", [D, D])               # w_q as [c, m]
    wkt_d = din("wkt", [D, D])
    wvt_d = din("wvt", [D, D])
    wo_d = din("wo", [D, D])                 # natural [m, c]
    w1t_d = din("w1t", [D, DFF])             # W1.T  [c, f]
    w2t_d = din("w2t", [DFF, D])             # W2.T  [f, c]
    onesr_d = din("onesr", [1, P])           # outer-product lhsT
    onesc_d = din("onesc", [P, 1])           # column-sum lhsT
    onesv_d = din("onesv", [P, 16 * H])      # ones column of V_ext
    masks_d = din("masks", [4, P, TB])       # triangular diag masks
    scalein_d = din("scalein", [P, 24], F32) # exp scale per (slot,chunk)
    biasin_d = din("biasin", [P, 24], F32)   # exp bias per (slot,chunk)
    g1_d = din("g1v", [D], F32)
    be1_d = din("be1v", [D], F32)
    g2_d = din("g2v", [D], F32)
    be2_d = din("be2v", [D], F32)
    b1_d = din("b1v", [DFF], F32)
    b2_d = din("b2v", [D], F32)

    outt_d = nc.dram_tensor("outt", [D, 1024], F32, kind="ExternalOutput").ap()

    xt_r = xt_d.rearrange("(j p) t -> p j t", p=P)

    with tile.TileContext(nc) as tc, ExitStack() as ctx, \
         nc.allow_low_precision(reason="fp32r intermediates are intended"):
        consts = ctx.enter_context(tc.tile_pool(name="consts", bufs=1))
        ps = ctx.enter_context(tc.tile_pool(name="ps", bufs=1, space="PSUM"))
        rows = ctx.enter_context(tc.tile_pool(name="rows", bufs=1))
        work = ctx.enter_context(tc.tile_pool(name="work", bufs=2))

        onesr_sb = consts.tile([1, P], F32R, tag="onesr")
        onesc_sb = consts.tile([P, 1], F32R, tag="onesc")
        scale_sb = consts.tile([P, 24], F32, tag="scalein")
        bias_sb = consts.tile([P, 24], F32, tag="biasin")
        g1_sb = consts.tile([P, NC], F32, tag="g1")
        be1_sb = consts.tile([P, NC], F32, tag="be1")
        g2_sb = consts.tile([P, NC], F32, tag="g2")
        be2_sb = consts.tile([P, NC], F32, tag="be2")
        b1_sb = consts.tile([P, NF], F32, tag="b1")
        b2_sb = consts.tile([P, NC], F32, tag="b2")
        nc.sync.dma_start(onesr_sb[:], onesr_d)
        nc.sync.dma_start(onesc_sb[:], onesc_d)
        nc.sync.dma_start(scale_sb[:], scalein_d)
        nc.sync.dma_start(bias_sb[:], biasin_d)
        for sb, d in ((g1_sb, g1_d), (be1_sb, be1_d), (g2_sb, g2_d),
                      (be2_sb, be2_d)):
            nc.sync.dma_start(sb[:], d.rearrange("(j p) -> p j", p=P))
        nc.sync.dma_start(b1_sb[:], b1_d.rearrange("(j p) -> p j", p=P))
        nc.sync.dma_start(b2_sb[:], b2_d.rearrange("(j p) -> p j", p=P))

        def layernorm_t(src_sb, dst_sb, tspan, g_sb, be_sb):
            """src/dst: [128, NC, tspan] fp32r; normalize each token column."""
            for tb in range(tspan // TB):
                sl = slice(tb * TB, (tb + 1) * TB)
                srow = ps.tile([1, 2 * TB], F32, tag="misc", name="srow")
                s1 = srow[:, 0:TB]
                s2 = srow[:, TB:2 * TB]
                for j in range(NC):
                    nc.tensor.matmul(s1, onesc_sb[:], src_sb[:, j, sl],
                                     start=(j == 0), stop=(j == NC - 1))
                for j in range(NC):
                    sq = work.tile([P, TB], F32R, tag="sq")
                    nc.vector.tensor_mul(sq[:], src_sb[:, j, sl], src_sb[:, j, sl])
                    nc.tensor.matmul(s2, onesc_sb[:], sq[:],
                                     start=(j == 0), stop=(j == NC - 1))
                mu = rows.tile([1, TB], F32, tag="mu")
                t = rows.tile([1, TB], F32, tag="tmp")
                r = rows.tile([1, TB], F32R, tag="r")
                mur = rows.tile([1, TB], F32R, tag="mur")
                nc.vector.tensor_scalar_mul(mu[:], s1, 1.0 / D)
                nc.vector.tensor_mul(t[:], mu[:], mu[:])
                # t = s2/D - mu^2  (variance), then sqrt(t + eps), recip
                nc.vector.scalar_tensor_tensor(t[:], s2, 1.0 / D, t[:],
                                               ALU.mult, ALU.subtract)
                nc.vector.tensor_scalar_add(t[:], t[:], EPS)
                nc.scalar.activation(t[:], t[:], AF.Sqrt)
                nc.vector.reciprocal(r[:], t[:])
                nc.vector.tensor_mul(mur[:], mu[:], r[:])
                bc = ps.tile([P, 2 * TB], F32, tag="misc", name="bc")
                bcr = bc[:, 0:TB]
                bcmur = bc[:, TB:2 * TB]
                nc.tensor.matmul(bcr, onesr_sb[:], r[:], start=True, stop=True)
                nc.tensor.matmul(bcmur, onesr_sb[:], mur[:], start=True, stop=True)
                for j in range(NC):
                    t1 = work.tile([P, TB], F32R, tag="nrm")
                    nc.vector.tensor_mul(t1[:], src_sb[:, j, sl], bcr)
                    nc.vector.tensor_sub(t1[:], t1[:], bcmur)
                    nc.vector.tensor_scalar(dst_sb[:, j, sl], t1[:],
                                            g_sb[:, j:j + 1], be_sb[:, j:j + 1],
                                            ALU.mult, ALU.add)

        # ---------------- Phase 1: load X^T, LayerNorm 1 ----------------
        es_xn1 = ExitStack()
        p_xn1 = es_xn1.enter_context(tc.tile_pool(name="p_xn1", bufs=1))
        xn1_sb = p_xn1.tile([P, NC, T], F32R, tag="xn1")
        with tc.tile_pool(name="p_xt", bufs=1) as p_xt:
            xt_sb = p_xt.tile([P, NC, T], F32R, tag="xt")
            nc.sync.dma_start(xt_sb[:], xt_r)
            layernorm_t(xt_sb, xn1_sb, T, g1_sb, be1_sb)

        # ---------------- Phase 2: QKV projections ----------------
        es_kqv = ExitStack()
        p_kqv = es_kqv.enter_context(tc.tile_pool(name="p_kqv", bufs=1, side="right"))
        kt_sb = p_kqv.tile([P, NC, T], F32R, tag="kt")      # K^T [m, s]
        qt_sb = p_kqv.tile([P, NC, 1024], F32R, tag="qt")   # Q^T [m, t_own]
        v_sb = p_kqv.tile([P, 16, H * 65], F32R, tag="v")   # V_ext [s, (h,65)]
        v_view = v_sb.rearrange("p s (h e) -> p s h e", e=65)

        with tc.tile_pool(name="p_wvt", bufs=1, side="right") as p_wvt:
            nc.sync.dma_start(
                v_view[:, :, :, 64:65],
                onesv_d.rearrange("p (s h e) -> p s h e", s=16, h=H))
            wvt_r = wvt_d.rearrange("(j p) m -> p j m", p=P)
            for half, fsl, w in ((0, slice(0, TB), TB),
                                 (1, slice(TB, D), D - TB)):
                wvt_sb = p_wvt.tile([P, NC, TB], F32R, tag="wvt")
                nc.sync.dma_start(wvt_sb[:, :, :w], wvt_r[:, :, fsl])
                for st in range(16):
                    ssl = slice(st * P, (st + 1) * P)
                    acc = ps.tile([P, TB], F32, tag="acc", bufs=2)
                    for j in range(NC):
                        nc.tensor.matmul(acc[:, :w], xn1_sb[:, j, ssl],
                                         wvt_sb[:, j, :w],
                                         start=(j == 0), stop=(j == NC - 1))
                    # scatter [P, w] into the 65-strided V_ext layout
                    src = acc[:, :w].rearrange("p (h e) -> p h e", e=64)
                    h0 = half * 8
                    nc.vector.tensor_copy(
                        v_view[:, st, h0:h0 + w // 64, 0:64], src)

        with tc.tile_pool(name="p_wstream", bufs=1, side="right") as p_wstream:
            for mt in range(NC):
                msl = slice(mt * P, (mt + 1) * P)
                wq_t = p_wstream.tile([P, NC, P], F32R, tag="wq", bufs=2)
                nc.sync.dma_start(
                    wq_t[:], wqt_d.rearrange("(j p) m -> p j m", p=P)[:, :, msl])
                for sl_i in range(2):
                    tsl = slice(sl_i * TB, (sl_i + 1) * TB)
                    acc = ps.tile([P, TB], F32, tag="acc", bufs=2)
                    for j in range(NC):
                        nc.tensor.matmul(acc[:], wq_t[:, j, :], xn1_sb[:, j, tsl],
                                         start=(j == 0), stop=(j == NC - 1))
                    nc.vector.tensor_copy(qt_sb[:, mt, tsl], acc[:])
                wk_t = p_wstream.tile([P, NC, P], F32R, tag="wk", bufs=2)
                nc.sync.dma_start(
                    wk_t[:], wkt_d.rearrange("(j p) m -> p j m", p=P)[:, :, msl])
                for tb in range(NTB):
                    tsl = slice(tb * TB, (tb + 1) * TB)
                    acc = ps.tile([P, TB], F32, tag="acc", bufs=2)
                    for j in range(NC):
                        nc.tensor.matmul(acc[:], wk_t[:, j, :], xn1_sb[:, j, tsl],
                                         start=(j == 0), stop=(j == NC - 1))
                    nc.vector.tensor_copy(kt_sb[:, mt, tsl], acc[:])

        es_xn1.close()  # xn1 no longer needed

        # ---------------- Phase 3: attention ----------------
        p_xp = ctx.enter_context(tc.tile_pool(name="p_xp", bufs=1))
        xp_sb = p_xp.tile([P, NC, 1024], F32R, tag="xp")
        es_yt = ExitStack()
        p_yt = es_yt.enter_context(tc.tile_pool(name="p_yt", bufs=2))
        yt_slots = []
        with tc.tile_pool(name="p_masks", bufs=1, side="right") as p_masks, \
             tc.tile_pool(name="p_e", bufs=3, side="right") as p_e:
            masks_sb = p_masks.tile([P, 4, TB], F32R, tag="masks")
            nc.sync.dma_start(masks_sb[:], masks_d.rearrange("o p t -> p o t"))
            for sl_i in range(2):
                yt_all = p_yt.tile([P, NC, TB], F32R, tag="yt_all")
                yt_slots.append(yt_all)
                qsl = slice(sl_i * TB, (sl_i + 1) * TB)
                chunks = SLOT_CHUNKS[sl_i]
                for mt in range(NC):
                    for ph in range(2):
                        o = ph * 64
                        h = 2 * mt + ph
                        yt_ps = ps.tile([65, TB], F32, tag="yt", bufs=2)
                        for ci, ch in enumerate(chunks):
                            sb_idx = (0 if sl_i == 0 else 8) + ci
                            st_ps = ps.tile([P, TB], F32, tag="st", bufs=2)
                            nc.tensor.matmul(
                                st_ps[:], kt_sb[o:o + 64, mt, ch * P:(ch + 1) * P],
                                qt_sb[o:o + 64, mt, qsl], start=True, stop=True)
                            e_sb = p_e.tile([P, TB], F32R, tag="e")
                            nc.scalar.activation(
                                e_sb[:], st_ps[:], AF.Exp,
                                bias=bias_sb[:, sb_idx:sb_idx + 1],
                                scale=scale_sb[:, sb_idx:sb_idx + 1])
                            # self-diagonal 512-block: slot0 ch 0-3, slot1 ch 4-7
                            di = ch - 4 * sl_i
                            if 0 <= di < 4:
                                nc.vector.tensor_mul(e_sb[:], e_sb[:],
                                                     masks_sb[:, di, :])
                            nc.tensor.matmul(
                                yt_ps[:], v_sb[:, ch, h * 65:(h + 1) * 65],
                                e_sb[:], start=(ci == 0),
                                stop=(ci == len(chunks) - 1))
                        rc = rows.tile([1, TB], F32R, tag="rc")
                        nc.vector.reciprocal(rc[:], yt_ps[64:65, :])
                        bc = ps.tile([P, 2 * TB], F32, tag="misc", name="abc")
                        nc.tensor.matmul(bc[0:64, 0:TB], onesr_sb[:, 0:64],
                                         rc[:], start=True, stop=True)
                        dst = yt_all[o:o + 64, mt, :]
                        nc.vector.tensor_copy(dst, yt_ps[0:64, :])
                        nc.vector.tensor_mul(dst, dst, bc[0:64, 0:TB])

        es_kqv.close()  # kt/qt/v no longer needed

        # ---------------- Phase 4: w_o projection + residual ----------------
        with tc.tile_pool(name="p_wo", bufs=1) as p_wo:
            wo_sb = p_wo.tile([P, NC, D], F32R, tag="wo")
            nc.sync.dma_start(wo_sb[:], wo_d.rearrange("(j p) m -> p j m", p=P))
            xo_sb = p_wo.tile([P, NC, 1024], F32R, tag="xo")
            nc.sync.dma_start(xo_sb[:], xt_r[:, :, 0:1024])
            for sl_i in range(2):
                yt_all = yt_slots[sl_i]
                for ct in range(NC):
                    ao = ps.tile([P, TB], F32, tag="acc", bufs=2)
                    for mc in range(NC):
                        nc.tensor.matmul(ao[:], wo_sb[:, mc, ct * P:(ct + 1) * P],
                                         yt_all[:, mc, :],
                                         start=(mc == 0), stop=(mc == NC - 1))
                    nc.vector.tensor_add(xp_sb[:, ct, sl_i * TB:(sl_i + 1) * TB],
                                         xo_sb[:, ct, sl_i * TB:(sl_i + 1) * TB],
                                         ao[:])

        es_yt.close()

        # ---------------- Phase 5: LayerNorm 2 ----------------
        p_xn2 = ctx.enter_context(tc.tile_pool(name="p_xn2", bufs=1))
        xn2_sb = p_xn2.tile([P, NC, 1024], F32R, tag="xn2")
        layernorm_t(xp_sb, xn2_sb, 1024, g2_sb, be2_sb)

        # ---------------- Phase 6: MLP ----------------
        w1t_r = w1t_d.rearrange("(j p) f -> p j f", p=P)
        w2t_r = w2t_d.rearrange("(f p) c -> p f c", p=P)
        outt_r = outt_d.rearrange("(j p) t -> p j t", p=P)
        with tc.tile_pool(name="p_h1", bufs=1) as p_h1, \
             tc.tile_pool(name="p_wmlp", bufs=3) as p_wmlp, \
             tc.tile_pool(name="p_out", bufs=2) as p_out:
            h1_sb = p_h1.tile([P, NF, 1024], F32R, tag="h1")
            for ft in range(NF):
                w1_t = p_wmlp.tile([P, NC, P], F32R, tag="w1")
                nc.sync.dma_start(w1_t[:], w1t_r[:, :, ft * P:(ft + 1) * P])
                for tb in range(2):
                    tsl = slice(tb * TB, (tb + 1) * TB)
                    hp = ps.tile([P, TB], F32, tag="acc", bufs=2)
                    for j in range(NC):
                        nc.tensor.matmul(hp[:], w1_t[:, j, :], xn2_sb[:, j, tsl],
                                         start=(j == 0), stop=(j == NC - 1))
                    nc.scalar.activation(h1_sb[:, ft, tsl], hp[:], AF.Relu,
                                         bias=b1_sb[:, ft:ft + 1])
            for g in range(2):
                for tb in range(2):
                    tsl = slice(tb * TB, (tb + 1) * TB)
                    # reuse attention-phase psum tags (dead by now): 8-bank cap
                    o2s = [ps.tile([P, TB], F32, tag=t, bufs=bu,
                                   name=f"o2_{g}_{tb}_{ci}")
                           for ci, (t, bu) in enumerate(
                               (("st", 2), ("yt", 2), ("misc", 1)))]
                    for ft in range(NF):
                        w2_t = p_wmlp.tile([P, D], F32R, tag="w2")
                        nc.sync.dma_start(w2_t[:], w2t_r[:, ft, :])
                        for ci in range(3):
                            ct = g * 3 + ci
                            nc.tensor.matmul(o2s[ci][:],
                                             w2_t[:, ct * P:(ct + 1) * P],
                                             h1_sb[:, ft, tsl],
                                             start=(ft == 0), stop=(ft == NF - 1))
                    ot = p_out.tile([P, 3, TB], F32, tag="ot")
                    for ci in range(3):
                        ct = g * 3 + ci
                        nc.vector.scalar_tensor_tensor(
                            ot[:, ci, :], o2s[ci][:], b2_sb[:, ct:ct + 1],
                            xp_sb[:, ct, tsl], ALU.add, ALU.add)
                    nc.sync.dma_start(outt_r[:, g * 3:(g + 1) * 3, tsl], ot[:])

    nc.compile()
    return nc


def _host_inputs(X, w_q, w_k, w_v, w_o, W1, b1, W2, b2, g1, be1, g2, be2):
    """Build the 8 per-core input dicts."""
    f32 = np.float32
    wqt = np.ascontiguousarray(np.asarray(w_q, f32).reshape(D, D).T)
    wkt = np.ascontiguousarray(np.asarray(w_k, f32).reshape(D, D).T)
    wvt = np.ascontiguousarray(np.asarray(w_v, f32).reshape(D, D).T)
    wo = np.ascontiguousarray(np.asarray(w_o, f32))
    w1t = np.ascontiguousarray(np.asarray(W1, f32).T)
    w2t = np.ascontiguousarray(np.asarray(W2, f32).T)
    onesr = np.ones((1, P), f32)
    onesc = np.ones((P, 1), f32)
    onesv = np.ones((P, 16 * H), f32)
    # 4 canonical self-diagonal masks: mask[k][s, t] = (128k + s <= t)
    masks = np.zeros((4, P, TB), f32)
    ar_s = np.arange(P)[:, None]
    ar_t = np.arange(TB)[None, :]
    for k in range(4):
        masks[k] = (128 * k + ar_s <= ar_t).astype(f32)

    # per-role exp scale/bias: 24 = 8 (slot0) + 16 (slot1) chunk positions
    sc = {}
    bi = {}
    for role in range(2):
        order = ROLE_ORDER[role]
        s = np.full((24,), 0.125, f32)
        b = np.zeros((24,), f32)
        for sl_i in range(2):
            own_blk = order[sl_i]
            for ci, ch in enumerate(SLOT_CHUNKS[sl_i]):
                idx = (0 if sl_i == 0 else 8) + ci
                pos = ch // 4           # permuted 512-block of this s-chunk
                blk = order[pos]
                if pos == sl_i or blk < own_blk:
                    pass                # diagonal (tri-masked) or past: live
                else:
                    s[idx] = 0.0        # future: dead
                    b[idx] = DEAD
        sc[role] = np.broadcast_to(s, (P, 24)).copy()
        bi[role] = np.broadcast_to(b, (P, 24)).copy()

    shared = dict(wqt=wqt, wkt=wkt, wvt=wvt, wo=wo, w1t=w1t, w2t=w2t,
                  onesr=onesr, onesc=onesc, onesv=onesv, masks=masks,
                  g1v=np.asarray(g1, f32), be1v=np.asarray(be1, f32),
                  g2v=np.asarray(g2, f32), be2v=np.asarray(be2, f32),
                  b1v=np.asarray(b1, f32), b2v=np.asarray(b2, f32))

    in_maps = []
    for core in range(8):
        role, b_idx = core // 4, core % 4
        order = ROLE_ORDER[role]
        xb = np.asarray(X[b_idx], f32)          # [T, D]
        xperm = np.concatenate([xb[o * TB:(o + 1) * TB] for o in order], axis=0)
        xt = np.ascontiguousarray(xperm.T)      # [D, T]
        m = dict(shared)
        m["xt"] = xt
        m["scalein"] = sc[role]
        m["biasin"] = bi[role]
        in_maps.append(m)
    return in_maps


def _assemble(results, dtype):
    out = np.empty((B, T, D), dtype)
    for core in range(8):
        role, b_idx = core // 4, core % 4
        order = ROLE_ORDER[role]
        ot = results[core]["outt"]              # [D, 1024]
        for sl_i in range(2):
            blk = order[sl_i]
            out[b_idx, blk * TB:(blk + 1) * TB] = \
                ot[:, sl_i * TB:(sl_i + 1) * TB].T
    return out


def kernel(X, w_q, w_k, w_v, w_o, W1, b1, W2, b2, g1, be1, g2, be2,
           _want_results=False, _trace=False):
    if "nc" not in _cached:
        _cached["nc"] = _build_nc()
    nc = _cached["nc"]
    in_maps = _host_inputs(X, w_q, w_k, w_v, w_o, W1, b1, W2, b2,
                           g1, be1, g2, be2)
    res = run_bass_kernel_spmd(nc, in_maps, core_ids=list(range(8)),
                               trace=_trace)
    out = _assemble(res.results, np.asarray(X).dtype)
    if _want_results:
        return out, res
    return out


# revision 16
# speedup vs baseline: 9005.3440x; 9005.3440x over previous
"""Trainium2 Bass kernel for a dense transformer block (pre-LN attn + MLP).

B=4, T=2048, D=768, H=12 (DH=64), DFF=3072, fp32.

Sharding: 8 cores = 4 batches x 2 roles. Each core processes one batch and
owns 1024 query tokens (two 512-blocks, paired {0,3}/{1,2} for causal load
balance). K/V are computed for the full 2048 tokens on both cores of a batch
(cheap), so there are NO collectives.

SPMD uniformity: all 8 cores run ONE identical NEFF. Causal structure is
carried in DATA, not code:
  - host permutes each batch's token axis to [own0, own1, otherA, otherB]
  - q-slot0 attends s-chunks {0..3, 8..11}; q-slot1 attends s-chunks {0..15}
  - per-(slot,chunk) exp scale/bias inputs select live / dead (zero) chunks
  - 4 canonical triangular masks handle the self-diagonal 512-blocks

Everything on-chip runs in a transposed layout (features on partitions,
tokens on the free axis) so no on-chip transposes are needed; all weight /
input transposes happen on the host in numpy. Matmuls run as float32r
(full PE speed, ~bf16x2 precision). LayerNorm statistics are computed with
ones-column matmuls; per-token stats are broadcast across partitions with
K=1 outer-product matmuls. Softmax denominators come for free from a ones
column appended to V (65-row PV matmul); the divide is folded in after PV.
"""

import sys

sys.path.insert(0, "/opt/trn_rl_repo")

from contextlib import ExitStack

import numpy as np

import concourse.bass as bass
import concourse.mybir as mybir
import concourse.tile as tile
from concourse import bacc
from concourse.bass_utils import run_bass_kernel_spmd

F32 = mybir.dt.float32
F32R = mybir.dt.float32r
AF = mybir.ActivationFunctionType
BF16 = mybir.dt.bfloat16
ALU = mybir.AluOpType

H, D, DFF = 12, 768, 3072
DH = 64
B, T = 4, 2048
EPS = 1e-5
P = 128
NC = D // P          # 6 feature chunks
NF = DFF // P        # 24 ff tiles
TB = 512             # token block
NTB = T // TB        # 4 blocks
SLOT_CHUNKS = [[0, 1, 2, 3, 8, 9, 10, 11], list(range(16))]
# role -> permuted block order [own0, own1, restA, restB] (original block ids)
ROLE_ORDER = [[0, 3, 1, 2], [1, 2, 0, 3]]
DEAD = -30000.0      # exp(DEAD) == 0 in fp32

_cached = {}
PHASE_MARKS = []


def _mark(nc, name):
    PHASE_MARKS.append((name, nc.next_id()))


def _build_nc():
    nc = bacc.Bacc("TRN2", target_bir_lowering=False, debug=False,
                   enable_asserts=False, num_devices=8)

    def din(name, shape, dt=F32R):
        return nc.dram_tensor(name, shape, dt, kind="ExternalInput").ap()

    xt_d = din("xt", [D, T])                 # X[b].T, token-permuted
    wqt_d = din("wqt", [D, D], BF16)         # w_q as [c, m]
    wkt_d = din("wkt", [D, D], BF16)
    wvt_d = din("wvt", [D, D], BF16)
    wo_d = din("wo", [D, D])                 # natural [m, c]
    w1t_d = din("w1t", [D, DFF], BF16)       # W1.T  [c, f]
    w2t_d = din("w2t", [DFF, D], BF16)       # W2.T  [f, c]
    onesr_d = din("onesr", [1, P])           # outer-product lhsT
    onesc_d = din("onesc", [P, 1])           # column-sum lhsT
    masks_d = din("masks", [4, P, 2 * TB], BF16)  # tri masks x2 halves
    scalein_d = din("scalein", [P, 24], F32) # exp scale per (slot,chunk)
    biasin_d = din("biasin", [P, 24], F32)   # exp bias per (slot,chunk)
    g1_d = din("g1v", [D], F32)
    be1_d = din("be1v", [D], F32)
    g2_d = din("g2v", [D], F32)
    be2_d = din("be2v", [D], F32)
    g1r_d = din("g1r", [1, D])               # gains as rows (outer lhs source)
    g2r_d = din("g2r", [1, D])
    b1_d = din("b1v", [DFF], F32)
    b2_d = din("b2v", [D], F32)

    outt_d = nc.dram_tensor("outt", [D, 1024], F32, kind="ExternalOutput").ap()

    xt_r = xt_d.rearrange("(j p) t -> p j t", p=P)

    with tile.TileContext(nc) as tc, ExitStack() as ctx, \
         nc.allow_low_precision(reason="fp32r/bf16 intermediates are intended"):
        consts = ctx.enter_context(tc.tile_pool(name="consts", bufs=1))
        ps = ctx.enter_context(tc.tile_pool(name="ps", bufs=1, space="PSUM"))
        rows = ctx.enter_context(tc.tile_pool(name="rows", bufs=1))
        work = ctx.enter_context(tc.tile_pool(name="work", bufs=2))

        onesr_sb = consts.tile([1, P], F32R, tag="onesr")
        onesc_sb = consts.tile([P, 1], F32R, tag="onesc")
        scale_sb = consts.tile([P, 24], F32, tag="scalein")
        bias_sb = consts.tile([P, 24], F32, tag="biasin")
        g1_sb = consts.tile([P, NC], F32, tag="g1")
        be1_sb = consts.tile([P, NC], F32, tag="be1")
        g2_sb = consts.tile([P, NC], F32, tag="g2")
        be2_sb = consts.tile([P, NC], F32, tag="be2")
        g1r_sb = consts.tile([1, D], F32R, tag="g1r")
        g2r_sb = consts.tile([1, D], F32R, tag="g2r")
        b1_sb = consts.tile([P, NF], F32, tag="b1")
        b2_sb = consts.tile([P, NC], F32, tag="b2")
        nc.sync.dma_start(onesr_sb[:], onesr_d)
        nc.sync.dma_start(onesc_sb[:], onesc_d)
        nc.sync.dma_start(scale_sb[:], scalein_d)
        nc.sync.dma_start(bias_sb[:], biasin_d)
        nc.sync.dma_start(g1r_sb[:], g1r_d)
        nc.sync.dma_start(g2r_sb[:], g2r_d)
        for sb, d in ((g1_sb, g1_d), (be1_sb, be1_d), (g2_sb, g2_d),
                      (be2_sb, be2_d)):
            nc.sync.dma_start(sb[:], d.rearrange("(j p) -> p j", p=P))
        nc.sync.dma_start(b1_sb[:], b1_d.rearrange("(j p) -> p j", p=P))
        nc.sync.dma_start(b2_sb[:], b2_d.rearrange("(j p) -> p j", p=P))

        def ln_stats(src_sl):
            """src_sl: [128, NC, TB] slice. Returns (r, mur) rows in SBUF."""
            s1 = ps.tile([1, TB], F32, tag="acc", bufs=2, name="s1")
            s2 = ps.tile([1, TB], F32, tag="acc", bufs=2, name="s2")
            for j in range(NC):
                nc.tensor.matmul(s1[:], onesc_sb[:], src_sl[:, j, :],
                                 start=(j == 0), stop=(j == NC - 1))
            for j in range(NC):
                sq = work.tile([P, TB], F32R, tag="sq")
                nc.scalar.activation(sq[:], src_sl[:, j, :], AF.Square)
                nc.tensor.matmul(s2[:], onesc_sb[:], sq[:],
                                 start=(j == 0), stop=(j == NC - 1))
            mu = rows.tile([1, TB], F32, tag="mu")
            t = rows.tile([1, TB], F32, tag="tmp")
            r = rows.tile([1, TB], F32R, tag="r")
            mur = rows.tile([1, TB], F32R, tag="mur")
            nc.vector.tensor_scalar_mul(mu[:], s1[:], 1.0 / D)
            nc.vector.tensor_mul(t[:], mu[:], mu[:])
            nc.vector.scalar_tensor_tensor(t[:], s2[:], 1.0 / D, t[:],
                                           ALU.mult, ALU.subtract)
            nc.vector.tensor_scalar_add(t[:], t[:], EPS)
            nc.scalar.activation(t[:], t[:], AF.Sqrt)
            nc.vector.reciprocal(r[:], t[:])
            nc.vector.tensor_mul(mur[:], mu[:], r[:])
            return r, mur

        def ln_normalize(src_sl, dst_sl, r, mur, g_sb, be_sb, g_row):
            """dst = ((src*g[p])*bc(r) + be[p]) - bc(g[p]*mur)."""
            bcr = ps.tile([P, TB], F32, tag="yt", bufs=2, name="bcs")[:]
            nc.tensor.matmul(bcr, onesr_sb[:], r[:], start=True, stop=True)
            for j in range(NC):
                bc2 = ps.tile([P, TB], F32, tag="acc", bufs=2, name="bc2")
                nc.tensor.matmul(bc2[:], g_row[:, j * P:(j + 1) * P], mur[:],
                                 start=True, stop=True)
                t1 = work.tile([P, TB], F32R, tag="nrm")
                nc.vector.scalar_tensor_tensor(t1[:], src_sl[:, j, :],
                                               g_sb[:, j:j + 1], bcr,
                                               ALU.mult, ALU.mult)
                nc.vector.scalar_tensor_tensor(dst_sl[:, j, :], t1[:],
                                               be_sb[:, j:j + 1], bc2[:],
                                               ALU.add, ALU.subtract)

        # ---------------- Phase 1+2: LN1 and QKV, software-pipelined --------
        _mark(nc, "ln1")
        es_xn1 = ExitStack()
        p_xn1 = es_xn1.enter_context(tc.tile_pool(name="p_xn1", bufs=1))
        xn1_sb = p_xn1.tile([P, NC, T], BF16, tag="xn1")

        es_kqv = ExitStack()
        p_kqv = es_kqv.enter_context(tc.tile_pool(name="p_kqv", bufs=1,
                                                  side="right"))
        kt_sb = p_kqv.tile([P, NC, T], BF16, tag="kt")      # K^T [m, s]
        qt_sb = p_kqv.tile([P, NC, 1024], BF16, tag="qt")   # Q^T [m, t_own]
        v_sb = p_kqv.tile([P, 16, H * 65], BF16, tag="v")   # V_ext [s, (h,65)]
        v_view = v_sb.rearrange("p s (h e) -> p s h e", e=65)
        nc.vector.memset(v_view[:, :, :, 64:65], 1.0)

        es_wqkv = ExitStack()
        p_wqkv = es_wqkv.enter_context(tc.tile_pool(name="p_wqkv", bufs=1,
                                                    side="right"))
        wq_sb = p_wqkv.tile([P, NC, D], BF16, tag="wq")
        wk_sb = p_wqkv.tile([P, NC, D], BF16, tag="wk")
        wv_sb = p_wqkv.tile([P, NC, D], BF16, tag="wv")

        def qkv_for_tb(tb):
            tsl = slice(tb * TB, (tb + 1) * TB)
            _mark(nc, "qkv")
            for mt in range(NC):
                msl = slice(mt * P, (mt + 1) * P)
                acc = ps.tile([P, TB], F32, tag="acc", bufs=2, name="ka")
                for j in range(NC):
                    nc.tensor.matmul(acc[:], wk_sb[:, j, msl], xn1_sb[:, j, tsl],
                                     start=(j == 0), stop=(j == NC - 1))
                if mt % 2 == 0:
                    nc.vector.tensor_copy(kt_sb[:, mt, tsl], acc[:])
                else:
                    nc.scalar.activation(kt_sb[:, mt, tsl], acc[:], AF.Copy)
            for st in range(tb * 4, tb * 4 + 4):
                ssl = slice(st * P, (st + 1) * P)
                for half, fsl, w in ((0, slice(0, TB), TB),
                                     (1, slice(TB, D), D - TB)):
                    acc = ps.tile([P, TB], F32, tag="acc", bufs=2, name="va")
                    for j in range(NC):
                        nc.tensor.matmul(acc[:, :w], xn1_sb[:, j, ssl],
                                         wv_sb[:, j, fsl],
                                         start=(j == 0), stop=(j == NC - 1))
                    src = acc[:, :w].rearrange("p (h e) -> p h e", e=64)
                    h0 = half * 8
                    nc.vector.tensor_copy(
                        v_view[:, st, h0:h0 + w // 64, 0:64], src)
            if tb < 2:
                for mt in range(NC):
                    msl = slice(mt * P, (mt + 1) * P)
                    acc = ps.tile([P, TB], F32, tag="acc", bufs=2, name="qa")
                    for j in range(NC):
                        nc.tensor.matmul(acc[:], wq_sb[:, j, msl],
                                         xn1_sb[:, j, tsl],
                                         start=(j == 0), stop=(j == NC - 1))
                    if mt % 2 == 0:
                        nc.vector.tensor_copy(qt_sb[:, mt, tsl], acc[:])
                    else:
                        nc.scalar.activation(qt_sb[:, mt, tsl], acc[:], AF.Copy)

        with tc.tile_pool(name="p_xtr", bufs=2) as p_xtr:
            stats = {}
            for tb in range(NTB):
                tsl = slice(tb * TB, (tb + 1) * TB)
                xt_t = p_xtr.tile([P, NC, TB], F32R, tag="xtr")
                nc.sync.dma_start(xt_t[:], xt_r[:, :, tsl])
                if tb == 0:
                    nc.sync.dma_start(wk_sb[:],
                                      wkt_d.rearrange("(j p) m -> p j m", p=P))
                    nc.sync.dma_start(wv_sb[:],
                                      wvt_d.rearrange("(j p) m -> p j m", p=P))
                    nc.sync.dma_start(wq_sb[:],
                                      wqt_d.rearrange("(j p) m -> p j m", p=P))
                stats[tb] = (xt_t, ln_stats(xt_t[:]))
                # pipeline: emit previous tb's normalize + QKV after this
                # tb's stats so PE has stats work during DVE normalize
                if tb > 0:
                    pt, (r, mur) = stats.pop(tb - 1)
                    ln_normalize(pt[:], xn1_sb[:, :, (tb - 1) * TB:tb * TB],
                                 r, mur, g1_sb, be1_sb, g1r_sb)
                    qkv_for_tb(tb - 1)
            pt, (r, mur) = stats.pop(NTB - 1)
            ln_normalize(pt[:], xn1_sb[:, :, (NTB - 1) * TB:NTB * TB],
                         r, mur, g1_sb, be1_sb, g1r_sb)
            qkv_for_tb(NTB - 1)
        es_wqkv.close()
        es_xn1.close()

        # ---------------- Phase 3: attention (+ per-slot wo/LN2) -----------
        _mark(nc, "attn")
        p_xp = ctx.enter_context(tc.tile_pool(name="p_xp", bufs=1))
        xp_sb = p_xp.tile([P, NC, 1024], F32R, tag="xp")
        p_xn2 = ctx.enter_context(tc.tile_pool(name="p_xn2", bufs=1))
        xn2_sb = p_xn2.tile([P, NC, 1024], BF16, tag="xn2")
        es_yt = ExitStack()
        p_yt = es_yt.enter_context(tc.tile_pool(name="p_yt", bufs=2))
        es_wo = ExitStack()
        p_wo = es_wo.enter_context(tc.tile_pool(name="p_wo", bufs=1))
        wo_sb = p_wo.tile([P, NC, D], F32R, tag="wo")
        nc.sync.dma_start(wo_sb[:], wo_d.rearrange("(j p) m -> p j m", p=P))
        xo_sb = p_wo.tile([P, NC, 1024], F32R, tag="xo")
        nc.sync.dma_start(xo_sb[:], xt_r[:, :, 0:1024])

        with tc.tile_pool(name="p_masks", bufs=1, side="right") as p_masks, \
             tc.tile_pool(name="p_e", bufs=3, side="right") as p_e:
            masks_sb = p_masks.tile([P, 4, 2 * TB], BF16, tag="masks")
            nc.sync.dma_start(masks_sb[:], masks_d.rearrange("o p t -> p o t"))
            for sl_i in range(2):
                yt_all = p_yt.tile([P, NC, TB], F32R, tag="yt_all")
                qsl = slice(sl_i * TB, (sl_i + 1) * TB)
                chunks = SLOT_CHUNKS[sl_i]
                for mt in range(NC):
                    yt2 = [ps.tile([65, TB], F32, tag="yt", bufs=2,
                                   name=f"yt_{sl_i}_{mt}_{ph}") for ph in range(2)]
                    for ci, ch in enumerate(chunks):
                        sb_idx = (0 if sl_i == 0 else 8) + ci
                        st2 = ps.tile([P, 2 * TB], F32, tag="st", bufs=2)
                        for ph in range(2):
                            o = ph * 64
                            nc.tensor.matmul(
                                st2[:, ph * TB:(ph + 1) * TB],
                                kt_sb[o:o + 64, mt, ch * P:(ch + 1) * P],
                                qt_sb[o:o + 64, mt, qsl], start=True, stop=True)
                        e_sb = p_e.tile([P, 2 * TB], BF16, tag="e")
                        nc.scalar.activation(
                            e_sb[:], st2[:], AF.Exp,
                            bias=bias_sb[:, sb_idx:sb_idx + 1],
                            scale=scale_sb[:, sb_idx:sb_idx + 1])
                        di = ch - 4 * sl_i
                        if 0 <= di < 4:
                            nc.vector.tensor_mul(e_sb[:], e_sb[:],
                                                 masks_sb[:, di, :])
                        for ph in range(2):
                            h = 2 * mt + ph
                            nc.tensor.matmul(
                                yt2[ph][:], v_sb[:, ch, h * 65:(h + 1) * 65],
                                e_sb[:, ph * TB:(ph + 1) * TB], start=(ci == 0),
                                stop=(ci == len(chunks) - 1))
                    for ph in range(2):
                        o = ph * 64
                        yt_ps = yt2[ph]
                        rc = rows.tile([1, TB], F32R, tag="rc")
                        nc.vector.reciprocal(rc[:], yt_ps[64:65, :])
                        bc = ps.tile([64, TB], F32, tag="acc", bufs=2,
                                     name="abc")
                        nc.tensor.matmul(bc[:], onesr_sb[:, 0:64],
                                         rc[:], start=True, stop=True)
                        dst = yt_all[o:o + 64, mt, :]
                        nc.vector.tensor_copy(dst, yt_ps[0:64, :])
                        nc.vector.tensor_mul(dst, dst, bc[:])
                # w_o projection + residual for this slot (fills exp-waits of
                # the other slot)
                _mark(nc, "wo")
                for ct in range(NC):
                    ao = ps.tile([P, TB], F32, tag="acc", bufs=2, name="ao")
                    for mc in range(NC):
                        nc.tensor.matmul(ao[:], wo_sb[:, mc, ct * P:(ct + 1) * P],
                                         yt_all[:, mc, :],
                                         start=(mc == 0), stop=(mc == NC - 1))
                    nc.vector.tensor_add(xp_sb[:, ct, qsl],
                                         xo_sb[:, ct, qsl], ao[:])
                _mark(nc, "ln2")
                r2, mur2 = ln_stats(xp_sb[:, :, qsl])
                ln_normalize(xp_sb[:, :, qsl], xn2_sb[:, :, qsl],
                             r2, mur2, g2_sb, be2_sb, g2r_sb)

        es_kqv.close()
        es_wo.close()
        es_yt.close()

        # ---------------- Phase 6: MLP ----------------
        _mark(nc, "mlp")
        w1t_r = w1t_d.rearrange("(j p) f -> p j f", p=P)
        w2t_r = w2t_d.rearrange("(f p) c -> p f c", p=P)
        outt_r = outt_d.rearrange("(j p) t -> p j t", p=P)
        with tc.tile_pool(name="p_h1", bufs=1) as p_h1, \
             tc.tile_pool(name="p_wmlp", bufs=3) as p_wmlp, \
             tc.tile_pool(name="p_out", bufs=2) as p_out:
            h1_sb = p_h1.tile([P, NF, 1024], BF16, tag="h1")
            for ft4 in range(NF // 4):
                w1_t = p_wmlp.tile([P, NC, 4 * P], BF16, tag="w1")
                nc.sync.dma_start(
                    w1_t[:], w1t_r[:, :, 4 * ft4 * P:(4 * ft4 + 4) * P])
                for sub in range(4):
                    ft = 4 * ft4 + sub
                    for tb in range(2):
                        tsl = slice(tb * TB, (tb + 1) * TB)
                        hp = ps.tile([P, TB], F32, tag="acc", bufs=2, name="hp")
                        for j in range(NC):
                            nc.tensor.matmul(hp[:],
                                             w1_t[:, j, sub * P:(sub + 1) * P],
                                             xn2_sb[:, j, tsl],
                                             start=(j == 0), stop=(j == NC - 1))
                        nc.scalar.activation(h1_sb[:, ft, tsl], hp[:], AF.Relu,
                                             bias=b1_sb[:, ft:ft + 1])
            for g in range(2):
                o2s = {}
                for ci, (t, bu) in enumerate((("acc", 2), ("yt", 2))):
                    for tb in range(2):
                        o2s[(ci, tb)] = ps.tile([P, TB], F32, tag=t, bufs=bu,
                                                name=f"o2_{g}_{ci}_{tb}")
                stp = ps.tile([P, 2 * TB], F32, tag="st", bufs=2,
                              name=f"o2st_{g}")
                o2s[(2, 0)] = stp[:, 0:TB]
                o2s[(2, 1)] = stp[:, TB:2 * TB]
                for ft4 in range(NF // 4):
                    w2_t = p_wmlp.tile([P, 4, D], BF16, tag="w2")
                    nc.sync.dma_start(w2_t[:],
                                      w2t_r[:, 4 * ft4:4 * ft4 + 4, :])
                    for sub in range(4):
                        ft = 4 * ft4 + sub
                        for ci in range(3):
                            ct = g * 3 + ci
                            for tb in range(2):
                                nc.tensor.matmul(
                                    o2s[(ci, tb)][:],
                                    w2_t[:, sub, ct * P:(ct + 1) * P],
                                    h1_sb[:, ft, tb * TB:(tb + 1) * TB],
                                    start=(ft == 0), stop=(ft == NF - 1))
                for tb in range(2):
                    tsl = slice(tb * TB, (tb + 1) * TB)
                    ot = p_out.tile([P, 3, TB], F32, tag="ot")
                    for ci in range(3):
                        ct = g * 3 + ci
                        nc.vector.scalar_tensor_tensor(
                            ot[:, ci, :], o2s[(ci, tb)][:], b2_sb[:, ct:ct + 1],
                            xp_sb[:, ct, tsl], ALU.add, ALU.add)
                    nc.sync.dma_start(outt_r[:, g * 3:(g + 1) * 3, tsl], ot[:])

    nc.compile()
    return nc


def _host_inputs(X, w_q, w_k, w_v, w_o, W1, b1, W2, b2, g1, be1, g2, be2):
    """Build the 8 per-core input dicts."""
    f32 = np.float32
    import ml_dtypes as _mld
    _bf = _mld.bfloat16
    wqt = np.ascontiguousarray(np.asarray(w_q, f32).reshape(D, D).T.astype(_bf))
    wkt = np.ascontiguousarray(np.asarray(w_k, f32).reshape(D, D).T.astype(_bf))
    wvt = np.ascontiguousarray(np.asarray(w_v, f32).reshape(D, D).T.astype(_bf))
    wo = np.ascontiguousarray(np.asarray(w_o, f32))
    w1t = None  # bf16, set below
    w2t = None  # bf16, set below
    onesr = np.ones((1, P), f32)
    onesc = np.ones((P, 1), f32)
    onesv = None  # set below after bf16 import
    # 4 canonical self-diagonal masks: mask[k][s, t] = (128k + s <= t)
    import ml_dtypes
    bf16 = ml_dtypes.bfloat16
    masks = np.zeros((4, P, 2 * TB), bf16)
    ar_s = np.arange(P)[:, None]
    ar_t = np.arange(TB)[None, :]
    for k in range(4):
        m = (128 * k + ar_s <= ar_t).astype(bf16)
        masks[k, :, 0:TB] = m
        masks[k, :, TB:2 * TB] = m
    w1t = np.ascontiguousarray(np.asarray(W1, f32).T.astype(bf16))
    w2t = np.ascontiguousarray(np.asarray(W2, f32).T.astype(bf16))

    # per-role exp scale/bias: 24 = 8 (slot0) + 16 (slot1) chunk positions
    sc = {}
    bi = {}
    for role in range(2):
        order = ROLE_ORDER[role]
        s = np.full((24,), 0.125, f32)
        b = np.zeros((24,), f32)
        for sl_i in range(2):
            own_blk = order[sl_i]
            for ci, ch in enumerate(SLOT_CHUNKS[sl_i]):
                idx = (0 if sl_i == 0 else 8) + ci
                pos = ch // 4           # permuted 512-block of this s-chunk
                blk = order[pos]
                if pos == sl_i or blk < own_blk:
                    pass                # diagonal (tri-masked) or past: live
                else:
                    s[idx] = 0.0        # future: dead
                    b[idx] = DEAD
        sc[role] = np.broadcast_to(s, (P, 24)).copy()
        bi[role] = np.broadcast_to(b, (P, 24)).copy()

    g1r = np.asarray(g1, f32).reshape(1, D)
    g2r = np.asarray(g2, f32).reshape(1, D)
    shared = dict(wqt=wqt, wkt=wkt, wvt=wvt, wo=wo, w1t=w1t, w2t=w2t,
                  g1r=g1r, g2r=g2r,
                  onesr=onesr, onesc=onesc, masks=masks,
                  g1v=np.asarray(g1, f32), be1v=np.asarray(be1, f32),
                  g2v=np.asarray(g2, f32), be2v=np.asarray(be2, f32),
                  b1v=np.asarray(b1, f32), b2v=np.asarray(b2, f32))

    in_maps = []
    for core in range(8):
        role, b_idx = core // 4, core % 4
        order = ROLE_ORDER[role]
        xb = np.asarray(X[b_idx], f32)          # [T, D]
        xperm = np.concatenate([xb[o * TB:(o + 1) * TB] for o in order], axis=0)
        xt = np.ascontiguousarray(xperm.T)      # [D, T]
        m = dict(shared)
        m["xt"] = xt
        m["scalein"] = sc[role]
        m["biasin"] = bi[role]
        in_maps.append(m)
    return in_maps


def _assemble(results, dtype):
    out = np.empty((B, T, D), dtype)
    for core in range(8):
        role, b_idx = core // 4, core % 4
        order = ROLE_ORDER[role]
        ot = results[core]["outt"]              # [D, 1024]
        for sl_i in range(2):
            blk = order[sl_i]
            out[b_idx, blk * TB:(blk + 1) * TB] = \
                ot[:, sl_i * TB:(sl_i + 1) * TB].T
    return out


def kernel(X, w_q, w_k, w_v, w_o, W1, b1, W2, b2, g1, be1, g2, be2,
           _want_results=False, _trace=False):
    if "nc" not in _cached:
        _cached["nc"] = _build_nc()
    nc = _cached["nc"]
    in_maps = _host_inputs(X, w_q, w_k, w_v, w_o, W1, b1, W2, b2,
                           g1, be1, g2, be2)
    res = run_bass_kernel_spmd(nc, in_maps, core_ids=list(range(8)),
                               trace=_trace)
    out = _assemble(res.results, np.asarray(X).dtype)
    if _want_results:
        return out, res
    return out


# revision 19
# speedup vs baseline: 9129.5461x; 1.0138x over previous
"""Trainium2 Bass kernel for a dense transformer block (pre-LN attn + MLP).

B=4, T=2048, D=768, H=12 (DH=64), DFF=3072, fp32.

Sharding: 8 cores = 4 batches x 2 roles. Each core processes one batch and
owns 1024 query tokens (two 512-blocks, paired {0,3}/{1,2} for causal load
balance). K/V are computed for the full 2048 tokens on both cores of a batch
(cheap), so there are NO collectives.

SPMD uniformity: all 8 cores run ONE identical NEFF. Causal structure is
carried in DATA, not code:
  - host permutes each batch's token axis to [own0, own1, otherA, otherB]
  - q-slot0 attends s-chunks {0..3, 8..11}; q-slot1 attends s-chunks {0..15}
  - per-(slot,chunk) exp scale/bias inputs select live / dead (zero) chunks
  - 4 canonical triangular masks handle the self-diagonal 512-blocks

Everything on-chip runs in a transposed layout (features on partitions,
tokens on the free axis) so no on-chip transposes are needed; all weight /
input transposes happen on the host in numpy. Matmuls run as float32r
(full PE speed, ~bf16x2 precision). LayerNorm statistics are computed with
ones-column matmuls; per-token stats are broadcast across partitions with
K=1 outer-product matmuls. Softmax denominators come for free from a ones
column appended to V (65-row PV matmul); the divide is folded in after PV.
"""

import sys

sys.path.insert(0, "/opt/trn_rl_repo")

from contextlib import ExitStack

import numpy as np

import concourse.bass as bass
import concourse.mybir as mybir
import concourse.tile as tile
from concourse import bacc
from concourse.bass_utils import run_bass_kernel_spmd

F32 = mybir.dt.float32
F32R = mybir.dt.float32r
AF = mybir.ActivationFunctionType
BF16 = mybir.dt.bfloat16
ALU = mybir.AluOpType

H, D, DFF = 12, 768, 3072
DH = 64
B, T = 4, 2048
EPS = 1e-5
P = 128
NC = D // P          # 6 feature chunks
NF = DFF // P        # 24 ff tiles
TB = 512             # token block
NTB = T // TB        # 4 blocks
SLOT_CHUNKS = [[0, 1, 2, 3, 8, 9, 10, 11], list(range(16))]
# role -> permuted block order [own0, own1, restA, restB] (original block ids)
ROLE_ORDER = [[0, 3, 1, 2], [1, 2, 0, 3]]
DEAD = -30000.0      # exp(DEAD) == 0 in fp32

_cached = {}
PHASE_MARKS = []


def _mark(nc, name):
    PHASE_MARKS.append((name, nc.next_id()))


def _build_nc():
    nc = bacc.Bacc("TRN2", target_bir_lowering=False, debug=False,
                   enable_asserts=False, num_devices=8)

    def din(name, shape, dt=F32R):
        return nc.dram_tensor(name, shape, dt, kind="ExternalInput").ap()

    xt_d = din("xt", [D, T])                 # X[b].T, token-permuted
    wqt_d = din("wqt", [D, D], BF16)         # w_q as [c, m]
    wkt_d = din("wkt", [D, D], BF16)
    wvt_d = din("wvt", [D, D], BF16)
    wo_d = din("wo", [D, D])                 # natural [m, c]
    w1t_d = din("w1t", [D, DFF], BF16)       # W1.T  [c, f]
    w2t_d = din("w2t", [DFF, D], BF16)       # W2.T  [f, c]
    onesr_d = din("onesr", [1, P])           # outer-product lhsT
    onesc_d = din("onesc", [P, 1])           # column-sum lhsT
    masks_d = din("masks", [4, P, 2 * TB], BF16)  # tri masks x2 halves
    scalein_d = din("scalein", [P, 24], F32) # exp scale per (slot,chunk)
    biasin_d = din("biasin", [P, 24], F32)   # exp bias per (slot,chunk)
    g1_d = din("g1v", [D], F32)
    be1_d = din("be1v", [D], F32)
    g2_d = din("g2v", [D], F32)
    be2_d = din("be2v", [D], F32)
    g1r_d = din("g1r", [1, D])               # gains as rows (outer lhs source)
    g2r_d = din("g2r", [1, D])
    b1_d = din("b1v", [DFF], F32)
    b2_d = din("b2v", [D], F32)

    outt_d = nc.dram_tensor("outt", [D, 1024], F32, kind="ExternalOutput").ap()

    xt_r = xt_d.rearrange("(j p) t -> p j t", p=P)

    with tile.TileContext(nc) as tc, ExitStack() as ctx, \
         nc.allow_low_precision(reason="fp32r/bf16 intermediates are intended"):
        consts = ctx.enter_context(tc.tile_pool(name="consts", bufs=1))
        ps = ctx.enter_context(tc.tile_pool(name="ps", bufs=1, space="PSUM"))
        rows = ctx.enter_context(tc.tile_pool(name="rows", bufs=1))
        work = ctx.enter_context(tc.tile_pool(name="work", bufs=2))

        onesr_sb = consts.tile([1, P], F32R, tag="onesr")
        onesc_sb = consts.tile([P, 1], F32R, tag="onesc")
        scale_sb = consts.tile([P, 24], F32, tag="scalein")
        bias_sb = consts.tile([P, 24], F32, tag="biasin")
        g1_sb = consts.tile([P, NC], F32, tag="g1")
        be1_sb = consts.tile([P, NC], F32, tag="be1")
        g2_sb = consts.tile([P, NC], F32, tag="g2")
        be2_sb = consts.tile([P, NC], F32, tag="be2")
        g1r_sb = consts.tile([1, D], F32R, tag="g1r")
        g2r_sb = consts.tile([1, D], F32R, tag="g2r")
        b1_sb = consts.tile([P, NF], F32, tag="b1")
        b2_sb = consts.tile([P, NC], F32, tag="b2")
        def _early_const_dmas():
            nc.sync.dma_start(onesc_sb[:], onesc_d)
            nc.sync.dma_start(onesr_sb[:], onesr_d)
            nc.sync.dma_start(g1r_sb[:], g1r_d)
            for sb, d in ((g1_sb, g1_d), (be1_sb, be1_d)):
                nc.sync.dma_start(sb[:], d.rearrange("(j p) -> p j", p=P))

        def _late_const_dmas():
            nc.sync.dma_start(scale_sb[:], scalein_d)
            nc.sync.dma_start(bias_sb[:], biasin_d)
            nc.sync.dma_start(g2r_sb[:], g2r_d)
            for sb, d in ((g2_sb, g2_d), (be2_sb, be2_d)):
                nc.sync.dma_start(sb[:], d.rearrange("(j p) -> p j", p=P))
            nc.sync.dma_start(b1_sb[:], b1_d.rearrange("(j p) -> p j", p=P))
            nc.sync.dma_start(b2_sb[:], b2_d.rearrange("(j p) -> p j", p=P))

        def ln_stats(src_sl):
            """src_sl: [128, NC, TB] slice. Returns (r, mur) rows in SBUF."""
            s1 = ps.tile([1, TB], F32, tag="acc", bufs=2, name="s1")
            s2 = ps.tile([1, TB], F32, tag="acc", bufs=2, name="s2")
            for j in range(NC):
                nc.tensor.matmul(s1[:], onesc_sb[:], src_sl[:, j, :],
                                 start=(j == 0), stop=(j == NC - 1))
            for j in range(NC):
                sq = work.tile([P, TB], F32R, tag="sq")
                nc.scalar.activation(sq[:], src_sl[:, j, :], AF.Square)
                nc.tensor.matmul(s2[:], onesc_sb[:], sq[:],
                                 start=(j == 0), stop=(j == NC - 1))
            mu = rows.tile([1, TB], F32, tag="mu")
            t = rows.tile([1, TB], F32, tag="tmp")
            r = rows.tile([1, TB], F32R, tag="r")
            mur = rows.tile([1, TB], F32R, tag="mur")
            nc.vector.tensor_scalar_mul(mu[:], s1[:], 1.0 / D)
            nc.vector.tensor_mul(t[:], mu[:], mu[:])
            nc.vector.scalar_tensor_tensor(t[:], s2[:], 1.0 / D, t[:],
                                           ALU.mult, ALU.subtract)
            nc.vector.tensor_scalar_add(t[:], t[:], EPS)
            nc.scalar.activation(t[:], t[:], AF.Sqrt)
            nc.vector.reciprocal(r[:], t[:])
            nc.vector.tensor_mul(mur[:], mu[:], r[:])
            return r, mur

        def ln_normalize(src_sl, dst_sl, r, mur, g_sb, be_sb, g_row):
            """dst = ((src*g[p])*bc(r) + be[p]) - bc(g[p]*mur)."""
            bcr = ps.tile([P, TB], F32, tag="yt", bufs=2, name="bcs")[:]
            nc.tensor.matmul(bcr, onesr_sb[:], r[:], start=True, stop=True)
            for j in range(NC):
                bc2 = ps.tile([P, TB], F32, tag="acc", bufs=2, name="bc2")
                nc.tensor.matmul(bc2[:], g_row[:, j * P:(j + 1) * P], mur[:],
                                 start=True, stop=True)
                t1 = work.tile([P, TB], F32R, tag="nrm")
                nc.vector.scalar_tensor_tensor(t1[:], src_sl[:, j, :],
                                               g_sb[:, j:j + 1], bcr,
                                               ALU.mult, ALU.mult)
                nc.vector.scalar_tensor_tensor(dst_sl[:, j, :], t1[:],
                                               be_sb[:, j:j + 1], bc2[:],
                                               ALU.add, ALU.subtract)

        # ---------------- Phase 1+2: LN1 and QKV, software-pipelined --------
        _mark(nc, "ln1")
        es_xn1 = ExitStack()
        p_xn1 = es_xn1.enter_context(tc.tile_pool(name="p_xn1", bufs=1))
        xn1_sb = p_xn1.tile([P, NC, T], BF16, tag="xn1")

        es_kqv = ExitStack()
        p_kqv = es_kqv.enter_context(tc.tile_pool(name="p_kqv", bufs=1,
                                                  side="right"))
        kt_sb = p_kqv.tile([P, NC, T], BF16, tag="kt")      # K^T [m, s]
        qt_sb = p_kqv.tile([P, NC, 1024], BF16, tag="qt")   # Q^T [m, t_own]
        v_sb = p_kqv.tile([P, 16, H * 65], BF16, tag="v")   # V_ext [s, (h,65)]
        v_view = v_sb.rearrange("p s (h e) -> p s h e", e=65)
        nc.vector.memset(v_view[:, :, :, 64:65], 1.0)

        es_wqkv = ExitStack()
        p_wqkv = es_wqkv.enter_context(tc.tile_pool(name="p_wqkv", bufs=1,
                                                    side="right"))
        wq_sb = p_wqkv.tile([P, NC, D], BF16, tag="wq")
        wk_sb = p_wqkv.tile([P, NC, D], BF16, tag="wk")
        wv_sb = p_wqkv.tile([P, NC, D], BF16, tag="wv")

        def qkv_for_tb(tb):
            tsl = slice(tb * TB, (tb + 1) * TB)
            _mark(nc, "qkv")
            for mt in range(NC):
                msl = slice(mt * P, (mt + 1) * P)
                acc = ps.tile([P, TB], F32, tag="acc", bufs=2, name="ka")
                for j in range(NC):
                    nc.tensor.matmul(acc[:], wk_sb[:, j, msl], xn1_sb[:, j, tsl],
                                     start=(j == 0), stop=(j == NC - 1))
                if mt % 2 == 0:
                    nc.vector.tensor_copy(kt_sb[:, mt, tsl], acc[:])
                else:
                    nc.scalar.activation(kt_sb[:, mt, tsl], acc[:], AF.Copy)
            for st in range(tb * 4, tb * 4 + 4):
                ssl = slice(st * P, (st + 1) * P)
                for half, fsl, w in ((0, slice(0, TB), TB),
                                     (1, slice(TB, D), D - TB)):
                    acc = ps.tile([P, TB], F32, tag="acc", bufs=2, name="va")
                    for j in range(NC):
                        nc.tensor.matmul(acc[:, :w], xn1_sb[:, j, ssl],
                                         wv_sb[:, j, fsl],
                                         start=(j == 0), stop=(j == NC - 1))
                    src = acc[:, :w].rearrange("p (h e) -> p h e", e=64)
                    h0 = half * 8
                    if st % 2 == 0:
                        nc.vector.tensor_copy(
                            v_view[:, st, h0:h0 + w // 64, 0:64], src)
                    else:
                        nc.scalar.activation(
                            v_view[:, st, h0:h0 + w // 64, 0:64], src, AF.Copy)
            if tb < 2:
                for mt in range(NC):
                    msl = slice(mt * P, (mt + 1) * P)
                    acc = ps.tile([P, TB], F32, tag="acc", bufs=2, name="qa")
                    for j in range(NC):
                        nc.tensor.matmul(acc[:], wq_sb[:, j, msl],
                                         xn1_sb[:, j, tsl],
                                         start=(j == 0), stop=(j == NC - 1))
                    if mt % 2 == 0:
                        nc.vector.tensor_copy(qt_sb[:, mt, tsl], acc[:])
                    else:
                        nc.scalar.activation(qt_sb[:, mt, tsl], acc[:], AF.Copy)

        with tc.tile_pool(name="p_xtr", bufs=2) as p_xtr:
            stats = {}
            for tb in range(NTB):
                tsl = slice(tb * TB, (tb + 1) * TB)
                xt_t = p_xtr.tile([P, NC, TB], F32R, tag="xtr")
                nc.sync.dma_start(xt_t[:], xt_r[:, :, tsl])
                if tb == 0:
                    _early_const_dmas()
                    nc.sync.dma_start(wk_sb[:],
                                      wkt_d.rearrange("(j p) m -> p j m", p=P))
                    nc.sync.dma_start(wv_sb[:],
                                      wvt_d.rearrange("(j p) m -> p j m", p=P))
                    nc.sync.dma_start(wq_sb[:],
                                      wqt_d.rearrange("(j p) m -> p j m", p=P))
                if tb == 1:
                    _late_const_dmas()
                stats[tb] = (xt_t, ln_stats(xt_t[:]))
                # pipeline: emit previous tb's normalize + QKV after this
                # tb's stats so PE has stats work during DVE normalize
                if tb > 0:
                    pt, (r, mur) = stats.pop(tb - 1)
                    ln_normalize(pt[:], xn1_sb[:, :, (tb - 1) * TB:tb * TB],
                                 r, mur, g1_sb, be1_sb, g1r_sb)
                    qkv_for_tb(tb - 1)
            pt, (r, mur) = stats.pop(NTB - 1)
            ln_normalize(pt[:], xn1_sb[:, :, (NTB - 1) * TB:NTB * TB],
                         r, mur, g1_sb, be1_sb, g1r_sb)
            qkv_for_tb(NTB - 1)
        es_wqkv.close()
        es_xn1.close()

        # ---------------- Phase 3: attention (+ per-slot wo/LN2) -----------
        _mark(nc, "attn")
        p_xp = ctx.enter_context(tc.tile_pool(name="p_xp", bufs=1))
        xp_sb = p_xp.tile([P, NC, 1024], F32R, tag="xp")
        p_xn2 = ctx.enter_context(tc.tile_pool(name="p_xn2", bufs=1))
        xn2_sb = p_xn2.tile([P, NC, 1024], BF16, tag="xn2")
        es_yt = ExitStack()
        p_yt = es_yt.enter_context(tc.tile_pool(name="p_yt", bufs=2))
        es_wo = ExitStack()
        p_wo = es_wo.enter_context(tc.tile_pool(name="p_wo", bufs=1))
        wo_sb = p_wo.tile([P, NC, D], F32R, tag="wo")
        nc.sync.dma_start(wo_sb[:], wo_d.rearrange("(j p) m -> p j m", p=P))
        xo_sb = p_wo.tile([P, NC, 1024], F32R, tag="xo")
        nc.sync.dma_start(xo_sb[:], xt_r[:, :, 0:1024])

        with tc.tile_pool(name="p_masks", bufs=1, side="right") as p_masks, \
             tc.tile_pool(name="p_e", bufs=3, side="right") as p_e:
            masks_sb = p_masks.tile([P, 4, 2 * TB], BF16, tag="masks")
            nc.sync.dma_start(masks_sb[:], masks_d.rearrange("o p t -> p o t"))
            for sl_i in range(2):
                yt_all = p_yt.tile([P, NC, TB], F32R, tag="yt_all")
                qsl = slice(sl_i * TB, (sl_i + 1) * TB)
                chunks = SLOT_CHUNKS[sl_i]
                for mt in range(NC):
                    yt2 = [ps.tile([65, TB], F32, tag="yt", bufs=2,
                                   name=f"yt_{sl_i}_{mt}_{ph}") for ph in range(2)]
                    for ci, ch in enumerate(chunks):
                        sb_idx = (0 if sl_i == 0 else 8) + ci
                        st2 = ps.tile([P, 2 * TB], F32, tag="st", bufs=2)
                        for ph in range(2):
                            o = ph * 64
                            nc.tensor.matmul(
                                st2[:, ph * TB:(ph + 1) * TB],
                                kt_sb[o:o + 64, mt, ch * P:(ch + 1) * P],
                                qt_sb[o:o + 64, mt, qsl], start=True, stop=True)
                        e_sb = p_e.tile([P, 2 * TB], BF16, tag="e")
                        nc.scalar.activation(
                            e_sb[:], st2[:], AF.Exp,
                            bias=bias_sb[:, sb_idx:sb_idx + 1],
                            scale=scale_sb[:, sb_idx:sb_idx + 1])
                        di = ch - 4 * sl_i
                        if 0 <= di < 4:
                            nc.vector.tensor_mul(e_sb[:], e_sb[:],
                                                 masks_sb[:, di, :])
                        for ph in range(2):
                            h = 2 * mt + ph
                            nc.tensor.matmul(
                                yt2[ph][:], v_sb[:, ch, h * 65:(h + 1) * 65],
                                e_sb[:, ph * TB:(ph + 1) * TB], start=(ci == 0),
                                stop=(ci == len(chunks) - 1))
                    for ph in range(2):
                        o = ph * 64
                        yt_ps = yt2[ph]
                        rc = rows.tile([1, TB], F32R, tag="rc")
                        nc.vector.reciprocal(rc[:], yt_ps[64:65, :])
                        bc = ps.tile([64, TB], F32, tag="acc", bufs=2,
                                     name="abc")
                        nc.tensor.matmul(bc[:], onesr_sb[:, 0:64],
                                         rc[:], start=True, stop=True)
                        dst = yt_all[o:o + 64, mt, :]
                        nc.vector.tensor_copy(dst, yt_ps[0:64, :])
                        nc.vector.tensor_mul(dst, dst, bc[:])
                # w_o projection + residual for this slot (fills exp-waits of
                # the other slot)
                _mark(nc, "wo")
                for ct in range(NC):
                    ao = ps.tile([P, TB], F32, tag="acc", bufs=2, name="ao")
                    for mc in range(NC):
                        nc.tensor.matmul(ao[:], wo_sb[:, mc, ct * P:(ct + 1) * P],
                                         yt_all[:, mc, :],
                                         start=(mc == 0), stop=(mc == NC - 1))
                    nc.vector.tensor_add(xp_sb[:, ct, qsl],
                                         xo_sb[:, ct, qsl], ao[:])
                _mark(nc, "ln2")
                r2, mur2 = ln_stats(xp_sb[:, :, qsl])
                ln_normalize(xp_sb[:, :, qsl], xn2_sb[:, :, qsl],
                             r2, mur2, g2_sb, be2_sb, g2r_sb)

        es_kqv.close()
        es_wo.close()
        es_yt.close()

        # ---------------- Phase 6: MLP ----------------
        _mark(nc, "mlp")
        w1t_r = w1t_d.rearrange("(j p) f -> p j f", p=P)
        w2t_r = w2t_d.rearrange("(f p) c -> p f c", p=P)
        outt_r = outt_d.rearrange("(j p) t -> p j t", p=P)
        with tc.tile_pool(name="p_h1", bufs=1) as p_h1, \
             tc.tile_pool(name="p_wmlp", bufs=3) as p_wmlp, \
             tc.tile_pool(name="p_out", bufs=2) as p_out:
            h1_sb = p_h1.tile([P, NF, 1024], BF16, tag="h1")
            for ft4 in range(NF // 4):
                w1_t = p_wmlp.tile([P, NC, 4 * P], BF16, tag="w1")
                nc.sync.dma_start(
                    w1_t[:], w1t_r[:, :, 4 * ft4 * P:(4 * ft4 + 4) * P])
                for sub in range(4):
                    ft = 4 * ft4 + sub
                    for tb in range(2):
                        tsl = slice(tb * TB, (tb + 1) * TB)
                        hp = ps.tile([P, TB], F32, tag="acc", bufs=2, name="hp")
                        for j in range(NC):
                            nc.tensor.matmul(hp[:],
                                             w1_t[:, j, sub * P:(sub + 1) * P],
                                             xn2_sb[:, j, tsl],
                                             start=(j == 0), stop=(j == NC - 1))
                        nc.scalar.activation(h1_sb[:, ft, tsl], hp[:], AF.Relu,
                                             bias=b1_sb[:, ft:ft + 1])
            for g in range(2):
                o2s = {}
                for ci, (t, bu) in enumerate((("acc", 2), ("yt", 2))):
                    for tb in range(2):
                        o2s[(ci, tb)] = ps.tile([P, TB], F32, tag=t, bufs=bu,
                                                name=f"o2_{g}_{ci}_{tb}")
                stp = ps.tile([P, 2 * TB], F32, tag="st", bufs=2,
                              name=f"o2st_{g}")
                o2s[(2, 0)] = stp[:, 0:TB]
                o2s[(2, 1)] = stp[:, TB:2 * TB]
                for ft4 in range(NF // 4):
                    w2_t = p_wmlp.tile([P, 4, D], BF16, tag="w2")
                    nc.sync.dma_start(w2_t[:],
                                      w2t_r[:, 4 * ft4:4 * ft4 + 4, :])
                    for sub in range(4):
                        ft = 4 * ft4 + sub
                        for ci in range(3):
                            ct = g * 3 + ci
                            for tb in range(2):
                                nc.tensor.matmul(
                                    o2s[(ci, tb)][:],
                                    w2_t[:, sub, ct * P:(ct + 1) * P],
                                    h1_sb[:, ft, tb * TB:(tb + 1) * TB],
                                    start=(ft == 0), stop=(ft == NF - 1))
                for tb in range(2):
                    tsl = slice(tb * TB, (tb + 1) * TB)
                    ot = p_out.tile([P, 3, TB], F32, tag="ot")
                    for ci in range(3):
                        ct = g * 3 + ci
                        nc.vector.scalar_tensor_tensor(
                            ot[:, ci, :], o2s[(ci, tb)][:], b2_sb[:, ct:ct + 1],
                            xp_sb[:, ct, tsl], ALU.add, ALU.add)
                    nc.sync.dma_start(outt_r[:, g * 3:(g + 1) * 3, tsl], ot[:])

    nc.compile()
    return nc


def _host_inputs(X, w_q, w_k, w_v, w_o, W1, b1, W2, b2, g1, be1, g2, be2):
    """Build the 8 per-core input dicts."""
    f32 = np.float32
    import ml_dtypes as _mld
    _bf = _mld.bfloat16
    wqt = np.ascontiguousarray(np.asarray(w_q, f32).reshape(D, D).T.astype(_bf))
    wkt = np.ascontiguousarray(np.asarray(w_k, f32).reshape(D, D).T.astype(_bf))
    wvt = np.ascontiguousarray(np.asarray(w_v, f32).reshape(D, D).T.astype(_bf))
    wo = np.ascontiguousarray(np.asarray(w_o, f32))
    w1t = None  # bf16, set below
    w2t = None  # bf16, set below
    onesr = np.ones((1, P), f32)
    onesc = np.ones((P, 1), f32)
    onesv = None  # set below after bf16 import
    # 4 canonical self-diagonal masks: mask[k][s, t] = (128k + s <= t)
    import ml_dtypes
    bf16 = ml_dtypes.bfloat16
    masks = np.zeros((4, P, 2 * TB), bf16)
    ar_s = np.arange(P)[:, None]
    ar_t = np.arange(TB)[None, :]
    for k in range(4):
        m = (128 * k + ar_s <= ar_t).astype(bf16)
        masks[k, :, 0:TB] = m
        masks[k, :, TB:2 * TB] = m
    w1t = np.ascontiguousarray(np.asarray(W1, f32).T.astype(bf16))
    w2t = np.ascontiguousarray(np.asarray(W2, f32).T.astype(bf16))

    # per-role exp scale/bias: 24 = 8 (slot0) + 16 (slot1) chunk positions
    sc = {}
    bi = {}
    for role in range(2):
        order = ROLE_ORDER[role]
        s = np.full((24,), 0.125, f32)
        b = np.zeros((24,), f32)
        for sl_i in range(2):
            own_blk = order[sl_i]
            for ci, ch in enumerate(SLOT_CHUNKS[sl_i]):
                idx = (0 if sl_i == 0 else 8) + ci
                pos = ch // 4           # permuted 512-block of this s-chunk
                blk = order[pos]
                if pos == sl_i or blk < own_blk:
                    pass                # diagonal (tri-masked) or past: live
                else:
                    s[idx] = 0.0        # future: dead
                    b[idx] = DEAD
        sc[role] = np.broadcast_to(s, (P, 24)).copy()
        bi[role] = np.broadcast_to(b, (P, 24)).copy()

    g1r = np.asarray(g1, f32).reshape(1, D)
    g2r = np.asarray(g2, f32).reshape(1, D)
    shared = dict(wqt=wqt, wkt=wkt, wvt=wvt, wo=wo, w1t=w1t, w2t=w2t,
                  g1r=g1r, g2r=g2r,
                  onesr=onesr, onesc=onesc, masks=masks,
                  g1v=np.asarray(g1, f32), be1v=np.asarray(be1, f32),
                  g2v=np.asarray(g2, f32), be2v=np.asarray(be2, f32),
                  b1v=np.asarray(b1, f32), b2v=np.asarray(b2, f32))

    in_maps = []
    for core in range(8):
        role, b_idx = core // 4, core % 4
        order = ROLE_ORDER[role]
        xb = np.asarray(X[b_idx], f32)          # [T, D]
        xperm = np.concatenate([xb[o * TB:(o + 1) * TB] for o in order], axis=0)
        xt = np.ascontiguousarray(xperm.T)      # [D, T]
        m = dict(shared)
        m["xt"] = xt
        m["scalein"] = sc[role]
        m["biasin"] = bi[role]
        in_maps.append(m)
    return in_maps


def _assemble(results, dtype):
    out = np.empty((B, T, D), dtype)
    for core in range(8):
        role, b_idx = core // 4, core % 4
        order = ROLE_ORDER[role]
        ot = results[core]["outt"]              # [D, 1024]
        for sl_i in range(2):
            blk = order[sl_i]
            out[b_idx, blk * TB:(blk + 1) * TB] = \
                ot[:, sl_i * TB:(sl_i + 1) * TB].T
    return out


def kernel(X, w_q, w_k, w_v, w_o, W1, b1, W2, b2, g1, be1, g2, be2,
           _want_results=False, _trace=False):
    if "nc" not in _cached:
        _cached["nc"] = _build_nc()
    nc = _cached["nc"]
    in_maps = _host_inputs(X, w_q, w_k, w_v, w_o, W1, b1, W2, b2,
                           g1, be1, g2, be2)
    res = run_bass_kernel_spmd(nc, in_maps, core_ids=list(range(8)),
                               trace=_trace)
    out = _assemble(res.results, np.asarray(X).dtype)
    if _want_results:
        return out, res
    return out


# revision 22
# speedup vs baseline: 9608.8330x; 1.0525x over previous
"""Trainium2 Bass kernel for a dense transformer block (pre-LN attn + MLP).

B=4, T=2048, D=768, H=12 (DH=64), DFF=3072, fp32.

Sharding: 8 cores = 4 batches x 2 roles. Each core processes one batch and
owns 1024 query tokens (two 512-blocks, paired {0,3}/{1,2} for causal load
balance). K/V are computed for the full 2048 tokens on both cores of a batch
(cheap), so there are NO collectives.

SPMD uniformity: all 8 cores run ONE identical NEFF. Causal structure is
carried in DATA, not code:
  - host permutes each batch's token axis to [own0, own1, otherA, otherB]
  - q-slot0 attends s-chunks {0..3, 8..11}; q-slot1 attends s-chunks {0..15}
  - per-(slot,chunk) exp scale/bias inputs select live / dead (zero) chunks
  - 4 canonical triangular masks handle the self-diagonal 512-blocks

Everything on-chip runs in a transposed layout (features on partitions,
tokens on the free axis) so no on-chip transposes are needed; all weight /
input transposes happen on the host in numpy. Matmuls run as float32r
(full PE speed, ~bf16x2 precision). LayerNorm statistics are computed with
ones-column matmuls; per-token stats are broadcast across partitions with
K=1 outer-product matmuls. Softmax denominators come for free from a ones
column appended to V (65-row PV matmul); the divide is folded in after PV.
"""

import sys

sys.path.insert(0, "/opt/trn_rl_repo")

from contextlib import ExitStack

import numpy as np

import concourse.bass as bass
import concourse.mybir as mybir
import concourse.tile as tile
from concourse import bacc
from concourse.bass_utils import run_bass_kernel_spmd

F32 = mybir.dt.float32
F32R = mybir.dt.float32r
AF = mybir.ActivationFunctionType
BF16 = mybir.dt.bfloat16
ALU = mybir.AluOpType

H, D, DFF = 12, 768, 3072
DH = 64
B, T = 4, 2048
EPS = 1e-5
P = 128
NC = D // P          # 6 feature chunks
NF = DFF // P        # 24 ff tiles
TB = 512             # token block
NTB = T // TB        # 4 blocks
SLOT_CHUNKS = [[0, 1, 2, 3, 8, 9, 10, 11], list(range(16))]
# role -> permuted block order [own0, own1, restA, restB] (original block ids)
ROLE_ORDER = [[0, 3, 1, 2], [1, 2, 0, 3]]
DEAD = -30000.0      # exp(DEAD) == 0 in fp32

_cached = {}
PHASE_MARKS = []


def _mark(nc, name):
    PHASE_MARKS.append((name, nc.next_id()))


def _build_nc():
    nc = bacc.Bacc("TRN2", target_bir_lowering=False, debug=False,
                   enable_asserts=False, num_devices=8)

    def din(name, shape, dt=F32R):
        return nc.dram_tensor(name, shape, dt, kind="ExternalInput").ap()

    xt_d = din("xt", [D, T])                 # X[b].T, token-permuted
    wqt_d = din("wqt", [D, D], BF16)         # w_q as [c, m]
    wkt_d = din("wkt", [D, D], BF16)
    wvt_d = din("wvt", [D, D], BF16)
    wo_d = din("wo", [D, D])                 # natural [m, c]
    w1t_d = din("w1t", [D, DFF], BF16)       # W1.T  [c, f]
    w2t_d = din("w2t", [DFF, D], BF16)       # W2.T  [f, c]
    onesr_d = din("onesr", [1, P])           # outer-product lhsT
    onesc_d = din("onesc", [P, 1])           # column-sum lhsT
    masks_d = din("masks", [4, P, 2 * TB], BF16)  # tri masks x2 halves
    scalein_d = din("scalein", [P, 24], F32) # exp scale per (slot,chunk)
    biasin_d = din("biasin", [P, 24], F32)   # exp bias per (slot,chunk)
    g1_d = din("g1v", [D], F32)
    be1_d = din("be1v", [D], F32)
    g2_d = din("g2v", [D], F32)
    be2_d = din("be2v", [D], F32)
    g1r_d = din("g1r", [1, D])               # gains as rows (outer lhs source)
    g2r_d = din("g2r", [1, D])
    b1_d = din("b1v", [DFF], F32)
    b2_d = din("b2v", [D], F32)

    outt_d = nc.dram_tensor("outt", [D, 1024], F32, kind="ExternalOutput").ap()

    xt_r = xt_d.rearrange("(j p) t -> p j t", p=P)

    with tile.TileContext(nc) as tc, ExitStack() as ctx, \
         nc.allow_low_precision(reason="fp32r/bf16 intermediates are intended"):
        consts = ctx.enter_context(tc.tile_pool(name="consts", bufs=1))
        ps = ctx.enter_context(tc.tile_pool(name="ps", bufs=1, space="PSUM"))
        rows = ctx.enter_context(tc.tile_pool(name="rows", bufs=1))
        work = ctx.enter_context(tc.tile_pool(name="work", bufs=2))

        onesr_sb = consts.tile([1, P], F32R, tag="onesr")
        onesc_sb = consts.tile([P, 1], F32R, tag="onesc")
        scale_sb = consts.tile([P, 24], F32, tag="scalein")
        bias_sb = consts.tile([P, 24], F32, tag="biasin")
        g1_sb = consts.tile([P, NC], F32, tag="g1")
        be1_sb = consts.tile([P, NC], F32, tag="be1")
        g2_sb = consts.tile([P, NC], F32, tag="g2")
        be2_sb = consts.tile([P, NC], F32, tag="be2")
        g1r_sb = consts.tile([1, D], F32R, tag="g1r")
        g2r_sb = consts.tile([1, D], F32R, tag="g2r")
        b1_sb = consts.tile([P, NF], F32, tag="b1")
        b2_sb = consts.tile([P, NC], F32, tag="b2")
        def _early_const_dmas():
            nc.sync.dma_start(onesc_sb[:], onesc_d)
            nc.sync.dma_start(onesr_sb[:], onesr_d)
            nc.sync.dma_start(g1r_sb[:], g1r_d)
            for sb, d in ((g1_sb, g1_d), (be1_sb, be1_d)):
                nc.sync.dma_start(sb[:], d.rearrange("(j p) -> p j", p=P))

        def _late_const_dmas():
            nc.sync.dma_start(scale_sb[:], scalein_d)
            nc.sync.dma_start(bias_sb[:], biasin_d)
            nc.sync.dma_start(g2r_sb[:], g2r_d)
            for sb, d in ((g2_sb, g2_d), (be2_sb, be2_d)):
                nc.sync.dma_start(sb[:], d.rearrange("(j p) -> p j", p=P))
            nc.sync.dma_start(b1_sb[:], b1_d.rearrange("(j p) -> p j", p=P))
            nc.sync.dma_start(b2_sb[:], b2_d.rearrange("(j p) -> p j", p=P))
            nc.sync.dma_start(masks_sb[:], masks_d.rearrange("o p t -> p o t"))

        def ln_stats(src_sl):
            """src_sl: [128, NC, TB] slice. Returns (r, mur) rows in SBUF."""
            s1 = ps.tile([1, TB], F32, tag="acc", bufs=2, name="s1")
            s2 = ps.tile([1, TB], F32, tag="acc", bufs=2, name="s2")
            for j in range(NC):
                nc.tensor.matmul(s1[:], onesc_sb[:], src_sl[:, j, :],
                                 start=(j == 0), stop=(j == NC - 1))
            for j in range(NC):
                sq = work.tile([P, TB], F32R, tag="sq")
                nc.scalar.activation(sq[:], src_sl[:, j, :], AF.Square)
                nc.tensor.matmul(s2[:], onesc_sb[:], sq[:],
                                 start=(j == 0), stop=(j == NC - 1))
            mu = rows.tile([1, TB], F32, tag="mu")
            t = rows.tile([1, TB], F32, tag="tmp")
            r = rows.tile([1, TB], F32R, tag="r")
            mur = rows.tile([1, TB], F32R, tag="mur")
            nc.vector.tensor_scalar_mul(mu[:], s1[:], 1.0 / D)
            nc.vector.tensor_mul(t[:], mu[:], mu[:])
            nc.vector.scalar_tensor_tensor(t[:], s2[:], 1.0 / D, t[:],
                                           ALU.mult, ALU.subtract)
            nc.vector.tensor_scalar_add(t[:], t[:], EPS)
            nc.scalar.activation(t[:], t[:], AF.Sqrt)
            nc.vector.reciprocal(r[:], t[:])
            nc.vector.tensor_mul(mur[:], mu[:], r[:])
            return r, mur

        def ln_normalize(src_sl, dst_sl, r, mur, g_sb, be_sb, g_row):
            """dst = ((src*g[p])*bc(r) + be[p]) - bc(g[p]*mur)."""
            bcr = ps.tile([P, TB], F32, tag="yt", bufs=2, name="bcs")[:]
            nc.tensor.matmul(bcr, onesr_sb[:], r[:], start=True, stop=True)
            for j in range(NC):
                bc2 = ps.tile([P, TB], F32, tag="acc", bufs=2, name="bc2")
                nc.tensor.matmul(bc2[:], g_row[:, j * P:(j + 1) * P], mur[:],
                                 start=True, stop=True)
                t1 = work.tile([P, TB], F32R, tag="nrm")
                nc.vector.scalar_tensor_tensor(t1[:], src_sl[:, j, :],
                                               g_sb[:, j:j + 1], bcr,
                                               ALU.mult, ALU.mult)
                nc.vector.scalar_tensor_tensor(dst_sl[:, j, :], t1[:],
                                               be_sb[:, j:j + 1], bc2[:],
                                               ALU.add, ALU.subtract)

        # ---------------- Phase 1+2: LN1 and QKV, software-pipelined --------
        _mark(nc, "ln1")
        es_xn1 = ExitStack()
        p_xn1 = es_xn1.enter_context(tc.tile_pool(name="p_xn1", bufs=1))
        xn1_sb = p_xn1.tile([P, NC, T], BF16, tag="xn1")

        es_kqv = ExitStack()
        p_kqv = es_kqv.enter_context(tc.tile_pool(name="p_kqv", bufs=1,
                                                  side="right"))
        kt_sb = p_kqv.tile([P, NC, T], BF16, tag="kt")      # K^T [m, s]
        qt_sb = p_kqv.tile([P, NC, 1024], BF16, tag="qt")   # Q^T [m, t_own]
        v_sb = p_kqv.tile([P, 16, H * 65], BF16, tag="v")   # V_ext [s, (h,65)]
        v_view = v_sb.rearrange("p s (h e) -> p s h e", e=65)
        nc.vector.memset(v_view[:, :, :, 64:65], 1.0)

        es_masks = ExitStack()
        p_masks = es_masks.enter_context(tc.tile_pool(name="p_masks", bufs=1,
                                                      side="right"))
        p_e = es_masks.enter_context(tc.tile_pool(name="p_e", bufs=3,
                                                  side="right"))
        masks_sb = p_masks.tile([P, 4, 2 * TB], BF16, tag="masks")

        es_wqkv = ExitStack()
        p_wqkv = es_wqkv.enter_context(tc.tile_pool(name="p_wqkv", bufs=1,
                                                    side="right"))
        wq_sb = p_wqkv.tile([P, NC, D], BF16, tag="wq")
        wk_sb = p_wqkv.tile([P, NC, D], BF16, tag="wk")
        wv_sb = p_wqkv.tile([P, NC, D], BF16, tag="wv")

        def qkv_for_tb(tb):
            tsl = slice(tb * TB, (tb + 1) * TB)
            _mark(nc, "qkv")
            for mt in range(NC):
                msl = slice(mt * P, (mt + 1) * P)
                acc = ps.tile([P, TB], F32, tag="acc", bufs=2, name="ka")
                for j in range(NC):
                    nc.tensor.matmul(acc[:], wk_sb[:, j, msl], xn1_sb[:, j, tsl],
                                     start=(j == 0), stop=(j == NC - 1))
                if mt % 2 == 0:
                    nc.vector.tensor_copy(kt_sb[:, mt, tsl], acc[:])
                else:
                    nc.scalar.activation(kt_sb[:, mt, tsl], acc[:], AF.Copy)
            for st in range(tb * 4, tb * 4 + 4):
                ssl = slice(st * P, (st + 1) * P)
                for half, fsl, w in ((0, slice(0, TB), TB),
                                     (1, slice(TB, D), D - TB)):
                    acc = ps.tile([P, TB], F32, tag="acc", bufs=2, name="va")
                    for j in range(NC):
                        nc.tensor.matmul(acc[:, :w], xn1_sb[:, j, ssl],
                                         wv_sb[:, j, fsl],
                                         start=(j == 0), stop=(j == NC - 1))
                    src = acc[:, :w].rearrange("p (h e) -> p h e", e=64)
                    h0 = half * 8
                    if st % 2 == 0:
                        nc.vector.tensor_copy(
                            v_view[:, st, h0:h0 + w // 64, 0:64], src)
                    else:
                        nc.scalar.activation(
                            v_view[:, st, h0:h0 + w // 64, 0:64], src, AF.Copy)
            if tb < 2:
                for mt in range(NC):
                    msl = slice(mt * P, (mt + 1) * P)
                    acc = ps.tile([P, TB], F32, tag="acc", bufs=2, name="qa")
                    for j in range(NC):
                        nc.tensor.matmul(acc[:], wq_sb[:, j, msl],
                                         xn1_sb[:, j, tsl],
                                         start=(j == 0), stop=(j == NC - 1))
                    if mt % 2 == 0:
                        nc.vector.tensor_copy(qt_sb[:, mt, tsl], acc[:])
                    else:
                        nc.scalar.activation(qt_sb[:, mt, tsl], acc[:], AF.Copy)

        with tc.tile_pool(name="p_xtr", bufs=2) as p_xtr:
            stats = {}
            for tb in range(NTB):
                tsl = slice(tb * TB, (tb + 1) * TB)
                xt_t = p_xtr.tile([P, NC, TB], F32R, tag="xtr")
                nc.sync.dma_start(xt_t[:], xt_r[:, :, tsl])
                if tb == 0:
                    _early_const_dmas()
                    nc.sync.dma_start(wk_sb[:],
                                      wkt_d.rearrange("(j p) m -> p j m", p=P))
                    nc.sync.dma_start(wv_sb[:],
                                      wvt_d.rearrange("(j p) m -> p j m", p=P))
                    nc.sync.dma_start(wq_sb[:],
                                      wqt_d.rearrange("(j p) m -> p j m", p=P))
                if tb == 1:
                    _late_const_dmas()
                stats[tb] = (xt_t, ln_stats(xt_t[:]))
                # pipeline: emit previous tb's normalize + QKV after this
                # tb's stats so PE has stats work during DVE normalize
                if tb > 0:
                    pt, (r, mur) = stats.pop(tb - 1)
                    ln_normalize(pt[:], xn1_sb[:, :, (tb - 1) * TB:tb * TB],
                                 r, mur, g1_sb, be1_sb, g1r_sb)
                    qkv_for_tb(tb - 1)
            pt, (r, mur) = stats.pop(NTB - 1)
            ln_normalize(pt[:], xn1_sb[:, :, (NTB - 1) * TB:NTB * TB],
                         r, mur, g1_sb, be1_sb, g1r_sb)
            qkv_for_tb(NTB - 1)
        es_wqkv.close()
        es_xn1.close()

        # ---------------- Phase 3: attention (+ per-slot wo/LN2) -----------
        _mark(nc, "attn")
        p_xp = ctx.enter_context(tc.tile_pool(name="p_xp", bufs=1))
        xp_sb = p_xp.tile([P, NC, 1024], F32R, tag="xp")
        p_xn2 = ctx.enter_context(tc.tile_pool(name="p_xn2", bufs=1))
        xn2_sb = p_xn2.tile([P, NC, 1024], BF16, tag="xn2")
        es_yt = ExitStack()
        p_yt = es_yt.enter_context(tc.tile_pool(name="p_yt", bufs=2))
        es_wo = ExitStack()
        p_wo = es_wo.enter_context(tc.tile_pool(name="p_wo", bufs=1))
        wo_sb = p_wo.tile([P, NC, D], F32R, tag="wo")
        nc.sync.dma_start(wo_sb[:], wo_d.rearrange("(j p) m -> p j m", p=P))
        xo_sb = p_wo.tile([P, NC, 1024], F32R, tag="xo")
        nc.sync.dma_start(xo_sb[:], xt_r[:, :, 0:1024])

        if True:
            for sl_i in range(2):
                yt_all = p_yt.tile([P, NC, TB], F32R, tag="yt_all")
                qsl = slice(sl_i * TB, (sl_i + 1) * TB)
                chunks = SLOT_CHUNKS[sl_i]
                for mt in range(NC):
                    yt2 = [ps.tile([65, TB], F32, tag="yt", bufs=2,
                                   name=f"yt_{sl_i}_{mt}_{ph}") for ph in range(2)]
                    for ci, ch in enumerate(chunks):
                        sb_idx = (0 if sl_i == 0 else 8) + ci
                        st2 = ps.tile([P, 2 * TB], F32, tag="st", bufs=2)
                        for ph in range(2):
                            o = ph * 64
                            nc.tensor.matmul(
                                st2[:, ph * TB:(ph + 1) * TB],
                                kt_sb[o:o + 64, mt, ch * P:(ch + 1) * P],
                                qt_sb[o:o + 64, mt, qsl], start=True, stop=True)
                        e_sb = p_e.tile([P, 2 * TB], BF16, tag="e")
                        nc.scalar.activation(
                            e_sb[:], st2[:], AF.Exp,
                            bias=bias_sb[:, sb_idx:sb_idx + 1],
                            scale=scale_sb[:, sb_idx:sb_idx + 1])
                        di = ch - 4 * sl_i
                        if 0 <= di < 4:
                            nc.vector.tensor_mul(e_sb[:], e_sb[:],
                                                 masks_sb[:, di, :])
                        for ph in range(2):
                            h = 2 * mt + ph
                            nc.tensor.matmul(
                                yt2[ph][:], v_sb[:, ch, h * 65:(h + 1) * 65],
                                e_sb[:, ph * TB:(ph + 1) * TB], start=(ci == 0),
                                stop=(ci == len(chunks) - 1))
                    for ph in range(2):
                        o = ph * 64
                        yt_ps = yt2[ph]
                        rc = rows.tile([1, TB], F32R, tag="rc")
                        nc.vector.reciprocal(rc[:], yt_ps[64:65, :])
                        bc = ps.tile([64, TB], F32, tag="acc", bufs=2,
                                     name="abc")
                        nc.tensor.matmul(bc[:], onesr_sb[:, 0:64],
                                         rc[:], start=True, stop=True)
                        dst = yt_all[o:o + 64, mt, :]
                        nc.vector.tensor_copy(dst, yt_ps[0:64, :])
                        nc.vector.tensor_mul(dst, dst, bc[:])
                # w_o projection + residual for this slot (fills exp-waits of
                # the other slot)
                _mark(nc, "wo")
                for ct in range(NC):
                    ao = ps.tile([P, TB], F32, tag="acc", bufs=2, name="ao")
                    for mc in range(NC):
                        nc.tensor.matmul(ao[:], wo_sb[:, mc, ct * P:(ct + 1) * P],
                                         yt_all[:, mc, :],
                                         start=(mc == 0), stop=(mc == NC - 1))
                    nc.vector.tensor_add(xp_sb[:, ct, qsl],
                                         xo_sb[:, ct, qsl], ao[:])
                _mark(nc, "ln2")
                r2, mur2 = ln_stats(xp_sb[:, :, qsl])
                ln_normalize(xp_sb[:, :, qsl], xn2_sb[:, :, qsl],
                             r2, mur2, g2_sb, be2_sb, g2r_sb)

        es_masks.close()
        es_kqv.close()
        es_wo.close()
        es_yt.close()

        # ---------------- Phase 6: MLP ----------------
        _mark(nc, "mlp")
        w1t_r = w1t_d.rearrange("(j p) f -> p j f", p=P)
        w2t_r = w2t_d.rearrange("(f p) c -> p f c", p=P)
        outt_r = outt_d.rearrange("(j p) t -> p j t", p=P)
        with tc.tile_pool(name="p_h1", bufs=1) as p_h1, \
             tc.tile_pool(name="p_wmlp", bufs=3) as p_wmlp, \
             tc.tile_pool(name="p_out", bufs=2) as p_out:
            h1_sb = p_h1.tile([P, NF, 1024], BF16, tag="h1")
            for ft4 in range(NF // 4):
                w1_t = p_wmlp.tile([P, NC, 4 * P], BF16, tag="w1")
                nc.sync.dma_start(
                    w1_t[:], w1t_r[:, :, 4 * ft4 * P:(4 * ft4 + 4) * P])
                for sub in range(4):
                    ft = 4 * ft4 + sub
                    for tb in range(2):
                        tsl = slice(tb * TB, (tb + 1) * TB)
                        hp = ps.tile([P, TB], F32, tag="acc", bufs=2, name="hp")
                        for j in range(NC):
                            nc.tensor.matmul(hp[:],
                                             w1_t[:, j, sub * P:(sub + 1) * P],
                                             xn2_sb[:, j, tsl],
                                             start=(j == 0), stop=(j == NC - 1))
                        nc.scalar.activation(h1_sb[:, ft, tsl], hp[:], AF.Relu,
                                             bias=b1_sb[:, ft:ft + 1])
            for g in range(2):
                o2s = {}
                for ci, (t, bu) in enumerate((("acc", 2), ("yt", 2))):
                    for tb in range(2):
                        o2s[(ci, tb)] = ps.tile([P, TB], F32, tag=t, bufs=bu,
                                                name=f"o2_{g}_{ci}_{tb}")
                stp = ps.tile([P, 2 * TB], F32, tag="st", bufs=2,
                              name=f"o2st_{g}")
                o2s[(2, 0)] = stp[:, 0:TB]
                o2s[(2, 1)] = stp[:, TB:2 * TB]
                for ft4 in range(NF // 4):
                    w2_t = p_wmlp.tile([P, 4, D], BF16, tag="w2")
                    nc.sync.dma_start(w2_t[:],
                                      w2t_r[:, 4 * ft4:4 * ft4 + 4, :])
                    for sub in range(4):
                        ft = 4 * ft4 + sub
                        for ci in range(3):
                            ct = g * 3 + ci
                            for tb in range(2):
                                nc.tensor.matmul(
                                    o2s[(ci, tb)][:],
                                    w2_t[:, sub, ct * P:(ct + 1) * P],
                                    h1_sb[:, ft, tb * TB:(tb + 1) * TB],
                                    start=(ft == 0), stop=(ft == NF - 1))
                for tb in range(2):
                    tsl = slice(tb * TB, (tb + 1) * TB)
                    ot = p_out.tile([P, 3, TB], F32, tag="ot")
                    for ci in range(3):
                        ct = g * 3 + ci
                        nc.vector.scalar_tensor_tensor(
                            ot[:, ci, :], o2s[(ci, tb)][:], b2_sb[:, ct:ct + 1],
                            xp_sb[:, ct, tsl], ALU.add, ALU.add)
                    nc.sync.dma_start(outt_r[:, g * 3:(g + 1) * 3, tsl], ot[:])

    nc.compile()
    return nc


def _host_inputs(X, w_q, w_k, w_v, w_o, W1, b1, W2, b2, g1, be1, g2, be2):
    """Build the 8 per-core input dicts."""
    f32 = np.float32
    import ml_dtypes as _mld
    _bf = _mld.bfloat16
    wqt = np.ascontiguousarray(np.asarray(w_q, f32).reshape(D, D).T.astype(_bf))
    wkt = np.ascontiguousarray(np.asarray(w_k, f32).reshape(D, D).T.astype(_bf))
    wvt = np.ascontiguousarray(np.asarray(w_v, f32).reshape(D, D).T.astype(_bf))
    wo = np.ascontiguousarray(np.asarray(w_o, f32))
    w1t = None  # bf16, set below
    w2t = None  # bf16, set below
    onesr = np.ones((1, P), f32)
    onesc = np.ones((P, 1), f32)
    onesv = None  # set below after bf16 import
    # 4 canonical self-diagonal masks: mask[k][s, t] = (128k + s <= t)
    import ml_dtypes
    bf16 = ml_dtypes.bfloat16
    masks = np.zeros((4, P, 2 * TB), bf16)
    ar_s = np.arange(P)[:, None]
    ar_t = np.arange(TB)[None, :]
    for k in range(4):
        m = (128 * k + ar_s <= ar_t).astype(bf16)
        masks[k, :, 0:TB] = m
        masks[k, :, TB:2 * TB] = m
    w1t = np.ascontiguousarray(np.asarray(W1, f32).T.astype(bf16))
    w2t = np.ascontiguousarray(np.asarray(W2, f32).T.astype(bf16))

    # per-role exp scale/bias: 24 = 8 (slot0) + 16 (slot1) chunk positions
    sc = {}
    bi = {}
    for role in range(2):
        order = ROLE_ORDER[role]
        s = np.full((24,), 0.125, f32)
        b = np.zeros((24,), f32)
        for sl_i in range(2):
            own_blk = order[sl_i]
            for ci, ch in enumerate(SLOT_CHUNKS[sl_i]):
                idx = (0 if sl_i == 0 else 8) + ci
                pos = ch // 4           # permuted 512-block of this s-chunk
                blk = order[pos]
                if pos == sl_i or blk < own_blk:
                    pass                # diagonal (tri-masked) or past: live
                else:
                    s[idx] = 0.0        # future: dead
                    b[idx] = DEAD
        sc[role] = np.broadcast_to(s, (P, 24)).copy()
        bi[role] = np.broadcast_to(b, (P, 24)).copy()

    g1r = np.asarray(g1, f32).reshape(1, D)
    g2r = np.asarray(g2, f32).reshape(1, D)
    shared = dict(wqt=wqt, wkt=wkt, wvt=wvt, wo=wo, w1t=w1t, w2t=w2t,
                  g1r=g1r, g2r=g2r,
                  onesr=onesr, onesc=onesc, masks=masks,
                  g1v=np.asarray(g1, f32), be1v=np.asarray(be1, f32),
                  g2v=np.asarray(g2, f32), be2v=np.asarray(be2, f32),
                  b1v=np.asarray(b1, f32), b2v=np.asarray(b2, f32))

    in_maps = []
    for core in range(8):
        role, b_idx = core // 4, core % 4
        order = ROLE_ORDER[role]
        xb = np.asarray(X[b_idx], f32)          # [T, D]
        xperm = np.concatenate([xb[o * TB:(o + 1) * TB] for o in order], axis=0)
        xt = np.ascontiguousarray(xperm.T)      # [D, T]
        m = dict(shared)
        m["xt"] = xt
        m["scalein"] = sc[role]
        m["biasin"] = bi[role]
        in_maps.append(m)
    return in_maps


def _assemble(results, dtype):
    out = np.empty((B, T, D), dtype)
    for core in range(8):
        role, b_idx = core // 4, core % 4
        order = ROLE_ORDER[role]
        ot = results[core]["outt"]              # [D, 1024]
        for sl_i in range(2):
            blk = order[sl_i]
            out[b_idx, blk * TB:(blk + 1) * TB] = \
                ot[:, sl_i * TB:(sl_i + 1) * TB].T
    return out


def kernel(X, w_q, w_k, w_v, w_o, W1, b1, W2, b2, g1, be1, g2, be2,
           _want_results=False, _trace=False):
    if "nc" not in _cached:
        _cached["nc"] = _build_nc()
    nc = _cached["nc"]
    in_maps = _host_inputs(X, w_q, w_k, w_v, w_o, W1, b1, W2, b2,
                           g1, be1, g2, be2)
    res = run_bass_kernel_spmd(nc, in_maps, core_ids=list(range(8)),
                               trace=_trace)
    out = _assemble(res.results, np.asarray(X).dtype)
    if _want_results:
        return out, res
    return out
